# revision 23
# baseline (speedup 1.0000x reference)
"""Causal multi-head attention kernel for 8 trn2 NeuronCores.

Problem: x[2,2048,1024], 16 heads of dim 64, causal softmax(q k^T / sqrt(1024)) v,
then output projection. Sharding: data-parallel over batch (4 cores per batch),
tensor-parallel over heads (4 heads per core). Each core produces a partial
output (its heads' contribution through Wout); the host sums the 4 partials per
batch and adds b_out.

Per-core device program (SPMD):
  - x arrives pre-transposed (host) as xT [d, n] in bf16; all weights bf16.
  - Projections: qT/kT [dh on partitions, n free] per head-PAIR (head A on
    partitions 0..63, head B on 64..127), v natural [n on partitions] with a
    shared ones column-block per (nb, pair): [dataA(64) | ones(64) | dataB(64)]
    so each head's AV matmul lhsT is a 128-col window whose ones half makes the
    matmul also produce softmax row-sums (A: sums on partitions 64..127,
    B: sums on partitions 0..63).
  - Attention in (cp = 512-wide i-chunk, pair, jb = 128-wide j-block) steps:
    the two heads' S^T = kT^T.qT matmuls have K=64 and run CONCURRENTLY on the
    PE via row-group tiling (head A rows 0-63, head B rows 64-127), writing the
    two halves of a pair-packed pS [128, 1024] (2 PSUM banks). The causal mask
    of the diagonal 128x128 block is applied ON THE PE as an extra accumulation
    matmul (lhsT = strictly-upper -30000 tile, rhs = identity), so the
    QK->exp->AV chain never crosses to the DVE. One ACT instruction computes
    exp for both heads through a 2-range strided AP. AV accumulates per-head
    O^T (+row sums) in PSUM over jb, lagged DELAY steps behind the exp so the
    PE never stalls on ACT. Block-causality skips all j>i blocks.
  - Normalization reads O^T straight from PSUM: reciprocal_approx_fast on the
    row-sum half + one tensor_mul -> OT in bf16 (no full-rate DVE reciprocal,
    no intermediate copy).
  - Output projection (contraction over the 4 heads = 2 pair-accumulated
    matmuls per [128n, 512d] tile) and the q/k/v projections stream through
    the attention steps as gated fillers to keep the PE dense (HAM warm).
"""

import os

import numpy as np
import ml_dtypes

B, N, D, H = 2, 2048, 1024, 16
DH = D // H  # 64
SCALE = float(D) ** -0.5
NCORES = 8
HPC = 4  # heads per core
NP = 2  # head pairs per core
IC = 512  # i-chunk width
NB = N // 128  # 16 j blocks
NCP = N // IC  # 4 i-chunks
KT = D // 128  # 8 contraction tiles
VW = 256  # v cols per (nb, pair): ones(64) | dataA(64) | ones(64) | dataB(64)
# sums land on pO partitions 0..63 for BOTH heads (reciprocal_approx_fast only
# works at base_partition 0), data on partitions 64..127
DELAY = 2
MASKV = -30000.0

_cached = {}
_last_results = None


def _build_program():
    import concourse.bacc as bacc
    import concourse.mybir as mybir
    import concourse.tile as tile

    f32 = mybir.dt.float32
    bf16 = mybir.dt.bfloat16
    EXP = mybir.ActivationFunctionType.Exp

    nc = bacc.Bacc()

    xb = nc.dram_tensor("xb", [D, N], bf16, kind="ExternalInput")  # x^T
    wq = nc.dram_tensor("wq", [D, HPC * DH], bf16, kind="ExternalInput")
    wk = nc.dram_tensor("wk", [D, HPC * DH], bf16, kind="ExternalInput")
    wv = nc.dram_tensor("wv", [D, HPC * DH], bf16, kind="ExternalInput")
    wo = nc.dram_tensor("wo", [HPC * DH, D], bf16, kind="ExternalInput")
    tri = nc.dram_tensor("tri", [128, 128], bf16, kind="ExternalInput")
    outp = nc.dram_tensor("outp", [N, D], f32, kind="ExternalOutput")

    with tile.TileContext(nc) as tc:
        with (
            tc.tile_pool(name="const", bufs=1) as const_pool,
            tc.tile_pool(name="big", bufs=1) as big_pool,
            tc.tile_pool(name="pS", bufs=2, space="PSUM") as pS_pool,
            tc.tile_pool(name="pO", bufs=2, space="PSUM") as pO_pool,
            tc.tile_pool(name="pj", bufs=2, space="PSUM") as pj_pool,
            tc.tile_pool(name="att", bufs=4) as att_pool,
            tc.tile_pool(name="rec", bufs=4) as rec_pool,
            tc.tile_pool(name="osb", bufs=3) as osb_pool,
        ):
            # Dummy exp early: pulls the ~2.7us ACT table load off the
            # critical path (overlaps the initial DMA).
            warm = const_pool.tile([1, 8], f32, name="warm", tag="warm")
            nc.vector.memset(warm, 0.0)
            nc.scalar.activation(out=warm, in_=warm, func=EXP, scale=1.0)
            # ~3.5us of dummy matmuls while the input DMA runs: trips the HAM
            # activity window so the real stream starts at 2.4GHz, not 1.2
            wa = const_pool.tile([128, 512], bf16, name="wa", tag="wa")
            nc.vector.memset(wa, 0.0)
            for _ in range(9):
                pwarm = pj_pool.tile([128, 512], f32, name="pwarm", tag="pj")
                nc.tensor.matmul(pwarm, lhsT=wa[:, 0:128], rhs=wa, start=True, stop=True)

            # one DMA instruction per weight tensor: d-tile r lands at free-dim
            # group r of a single [128, KT*256] tile (issue cost on the sync
            # queue is ~0.6us per DMA instruction, so batching matters)
            wqa = const_pool.tile([128, KT * 256], bf16, name="wqa", tag="wqa")
            wka = const_pool.tile([128, KT * 256], bf16, name="wka", tag="wka")
            wva = const_pool.tile([128, KT * 256], bf16, name="wva", tag="wva")
            woa = const_pool.tile([128, NP * D], bf16, name="woa", tag="woa")
            nc.sync.dma_start(
                out=wva, in_=wv[:, :].rearrange("(r p) c -> p r c", r=KT)
            )
            xTall = big_pool.tile([128, KT * N], bf16, name="xTall", tag="xTall")
            xT = [xTall[:, N * r : N * (r + 1)] for r in range(KT)]
            xT4 = xTall.rearrange("p (r c) -> p r c", r=KT)
            # first column-quarter of x^T lands first so projections start early
            nc.sync.dma_start(
                out=xT4[:, :, 0:512],
                in_=xb[:, 0:512].rearrange("(r p) c -> p r c", r=KT),
            )
            nc.sync.dma_start(
                out=xT4[:, :, 512:1024],
                in_=xb[:, 512:1024].rearrange("(r p) c -> p r c", r=KT),
            )
            nc.sync.dma_start(
                out=wqa, in_=wq[:, :].rearrange("(r p) c -> p r c", r=KT)
            )
            nc.sync.dma_start(
                out=wka, in_=wk[:, :].rearrange("(r p) c -> p r c", r=KT)
            )
            tri_sb = const_pool.tile([128, 128], bf16, name="tri_sb", tag="tri_sb")
            nc.sync.dma_start(out=tri_sb, in_=tri[:, :])
            nc.sync.dma_start(
                out=xT4[:, :, 1024:2048],
                in_=xb[:, 1024:2048].rearrange("(r p) c -> p r c", r=KT),
            )
            nc.sync.dma_start(
                out=woa, in_=wo[:, :].rearrange("(p q) c -> q p c", p=NP)
            )
            wq_sb = [wqa[:, 256 * r : 256 * (r + 1)] for r in range(KT)]
            wk_sb = [wka[:, 256 * r : 256 * (r + 1)] for r in range(KT)]
            wv_sb = [wva[:, 256 * r : 256 * (r + 1)] for r in range(KT)]
            wo_sb = [woa[:, D * p : D * (p + 1)] for p in range(NP)]

            qT, kT_ = [], []
            for p in range(NP):
                qT.append(big_pool.tile([128, N], bf16, name=f"qT{p}", tag=f"qT{p}"))
                kT_.append(big_pool.tile([128, N], bf16, name=f"kT{p}", tag=f"kT{p}"))
            v_all = big_pool.tile([128, NB * NP * VW], bf16, name="v_all", tag="v_all")
            # ones for the row-sum trick; data cols overwritten by vproj copies
            nc.vector.memset(v_all, 1.0)
            OT = []
            for p in range(NP):
                OT.append(big_pool.tile([128, N], bf16, name=f"OT{p}", tag=f"OT{p}"))

            va8 = v_all.rearrange("p (n g c) -> p n g c", n=NB, g=8)

            def vproj_stream(nbs):
                for nb in nbs:
                    pv = pj_pool.tile([128, HPC * DH], f32, name="pv", tag="pj")
                    for r in range(KT):
                        nc.tensor.matmul(
                            pv,
                            lhsT=xT[r][:, 128 * nb : 128 * (nb + 1)],
                            rhs=wv_sb[r],
                            start=(r == 0),
                            stop=(r == KT - 1),
                        )
                    # head h data -> 64-col group 2h+1 (odd groups; evens stay ones)
                    pv4 = pv.rearrange("p (h c) -> p h c", h=HPC)
                    nc.vector.tensor_copy(out=va8[:, nb, 1::2, :], in_=pv4)
                    yield

            def qkproj_stream(p, cs):
                for c in cs:
                    sl = slice(IC * c, IC * (c + 1))
                    for w_sb, dst in ((wq_sb, qT[p]), (wk_sb, kT_[p])):
                        pq = pj_pool.tile([128, IC], f32, name="pq", tag="pj")
                        for r in range(KT):
                            nc.tensor.matmul(
                                pq,
                                lhsT=w_sb[r][:, 128 * p : 128 * (p + 1)],
                                rhs=xT[r][:, sl],
                                start=(r == 0),
                                stop=(r == KT - 1),
                            )
                        nc.vector.tensor_copy(out=dst[:, sl], in_=pq)
                        yield

            def outproj_stream(nbs):
                for nb in nbs:
                    nsl = slice(128 * nb, 128 * (nb + 1))
                    for s in range(2):
                        po = pj_pool.tile([128, 512], f32, name="po", tag="pj")
                        for p in range(NP):
                            nc.tensor.matmul(
                                po,
                                lhsT=OT[p][:, nsl],
                                rhs=wo_sb[p][:, 512 * s : 512 * (s + 1)],
                                start=(p == 0),
                                stop=(p == NP - 1),
                            )
                        ob = osb_pool.tile([128, 512], f32, name="ob", tag="osb")
                        # DVE only: a scalar.copy's sem-wait would head-of-line
                        # block the exp stream on the Scalar queue
                        nc.vector.tensor_copy(out=ob, in_=po)
                        nc.gpsimd.dma_start(out=outp[nsl, 512 * s : 512 * (s + 1)], in_=ob)
                        yield

            pend = []

            def drain(n):
                while len(pend) > n:
                    pend.pop(0)()

            def attention_stream():
                for cp in range(NCP):
                    for p in range(NP):
                        pO_A = pO_pool.tile([128, IC], f32, name=f"pOA{cp}{p}", tag="pO")
                        pO_B = pO_pool.tile([128, IC], f32, name=f"pOB{cp}{p}", tag="pO")
                        jmax = 4 * cp + 4
                        for jb in range(jmax):
                            o = max(0, 128 * jb - IC * cp)
                            jsl = slice(128 * jb, 128 * (jb + 1))
                            isl = slice(IC * cp + o, IC * (cp + 1))
                            pS = pS_pool.tile([128, 2 * IC], f32, name="pS", tag="pS")
                            pexp = att_pool.tile([128, 2 * IC], bf16, name="pexp", tag="pexp")
                            # S^T pair: K=64 each, concurrent via row groups
                            nc.tensor.matmul(
                                pS[:, o:IC],
                                lhsT=kT_[p][0:64, jsl],
                                rhs=qT[p][0:64, isl],
                                start=True,
                                stop=True,
                            )
                            nc.tensor.matmul(
                                pS[:, IC + o : 2 * IC],
                                lhsT=kT_[p][64:128, jsl],
                                rhs=qT[p][64:128, isl],
                                start=True,
                                stop=True,
                            )
                            # one exp for both heads: [128, 2, IC-o] strided AP
                            src = pS.rearrange("p (h w) -> p h w", h=2)[:, :, o:]
                            dst = pexp.rearrange("p (h w) -> p h w", h=2)[:, :, o:]
                            nc.scalar.activation(out=dst, in_=src, func=EXP, scale=SCALE)
                            if 128 * jb >= IC * cp:  # diagonal block: 0/1 mask
                                for half in range(2):
                                    hb = IC * half
                                    nc.vector.tensor_mul(
                                        pexp[:, hb + o : hb + o + 128],
                                        pexp[:, hb + o : hb + o + 128],
                                        tri_sb,
                                    )

                            def av_unit(p=p, jb=jb, o=o, jmax=jmax, pO_A=pO_A, pO_B=pO_B, pexp=pexp):
                                vo = 2 * VW * jb + VW * p
                                nc.tensor.matmul(
                                    pO_A[:, o:IC],
                                    lhsT=v_all[:, vo : vo + 128],
                                    rhs=pexp[:, o:IC],
                                    start=(jb == 0),
                                    stop=(jb == jmax - 1),
                                    skip_group_check=True,
                                )
                                nc.tensor.matmul(
                                    pO_B[:, o:IC],
                                    lhsT=v_all[:, vo + 128 : vo + 256],
                                    rhs=pexp[:, IC + o : 2 * IC],
                                    start=(jb == 0),
                                    stop=(jb == jmax - 1),
                                    skip_group_check=True,
                                )

                            pend.append(av_unit)
                            drain(DELAY)
                            yield

                        # normalize straight from PSUM; OT written in bf16
                        csl = slice(IC * cp, IC * (cp + 1))
                        rec_A = rec_pool.tile([64, IC], f32, name="recA", tag="rec")
                        rec_B = rec_pool.tile([64, IC], f32, name="recB", tag="rec")

                        def recip_a(pO_A=pO_A, rec_A=rec_A):
                            nc.vector.reciprocal_approx_fast(out=rec_A, in_=pO_A[0:64, :])

                        def mul_a(pO_A=pO_A, rec_A=rec_A, p=p, csl=csl):
                            nc.vector.tensor_mul(OT[p][0:64, csl], pO_A[64:128, :], rec_A)

                        def recip_b(pO_B=pO_B, rec_B=rec_B):
                            nc.vector.reciprocal_approx_fast(out=rec_B, in_=pO_B[0:64, :])

                        def mul_b(pO_B=pO_B, rec_B=rec_B, p=p, csl=csl):
                            nc.vector.tensor_mul(OT[p][64:128, csl], pO_B[64:128, :], rec_B)

                        pend.append(recip_a)
                        pend.append(mul_a)
                        pend.append(recip_b)
                        pend.append(mul_b)

            # ---- prologue: the minimum attention (cp0, pair0) needs ----
            for _ in vproj_stream(range(0, 1)):
                pass
            for _ in qkproj_stream(0, [0]):
                pass

            # ---- gated fillers pulled between attention steps ----
            fillers = [
                (0, vproj_stream(range(1, 4))),
                (0, qkproj_stream(1, [0])),
                (0, qkproj_stream(0, [1])),
                (0, qkproj_stream(1, [1])),
                (0, vproj_stream(range(4, 8))),
                (8, qkproj_stream(0, [2])),
                (8, qkproj_stream(1, [2])),
                (8, vproj_stream(range(8, 12))),
                (12, outproj_stream(range(0, 4))),
                (24, qkproj_stream(0, [3])),
                (24, qkproj_stream(1, [3])),
                (24, vproj_stream(range(12, 16))),
                (28, outproj_stream(range(4, 8))),
                (52, outproj_stream(range(8, 12))),
            ]

            def pull_filler(step):
                for i, (gate, gen) in enumerate(fillers):
                    if step < gate:
                        continue
                    if next(gen, "end") == "end":
                        fillers.pop(i)
                        continue
                    return True
                return False

            att = attention_stream()
            for step, _ in enumerate(att):
                pull_filler(step)
                if step < 8 or step >= 48:
                    pull_filler(step)
            drain(0)
            for _, gen in fillers:
                for _ in gen:
                    pass
            # pair 1's cp3 normalize is only issued by drain(0) above, so the
            # last output-projection blocks must stay in the epilogue. Stage
            # them in one wide SBUF tile and ship with a single DMA (per-DMA
            # issue costs ~0.6us on the queue, the dominant tail cost).
            obig = big_pool.tile([128, 4 * D], f32, name="obig", tag="obig")
            for i, nb in enumerate(range(12, 16)):
                nsl = slice(128 * nb, 128 * (nb + 1))
                for s in range(2):
                    po = pj_pool.tile([128, 512], f32, name="po", tag="pj")
                    for p in range(NP):
                        nc.tensor.matmul(
                            po,
                            lhsT=OT[p][:, nsl],
                            rhs=wo_sb[p][:, 512 * s : 512 * (s + 1)],
                            start=(p == 0),
                            stop=(p == NP - 1),
                        )
                    dst = obig[:, D * i + 512 * s : D * i + 512 * (s + 1)]
                    if s == 0:
                        nc.vector.tensor_copy(out=dst, in_=po)
                    else:
                        nc.scalar.copy(out=dst, in_=po)
                if nb == 13:
                    nc.sync.dma_start(
                        out=outp[1536:1792, :].rearrange("(n p) c -> p n c", n=2),
                        in_=obig[:, 0 : 2 * D].rearrange("p (n c) -> p n c", n=2),
                    )
            nc.sync.dma_start(
                out=outp[1792:2048, :].rearrange("(n p) c -> p n c", n=2),
                in_=obig[:, 2 * D : 4 * D].rearrange("p (n c) -> p n c", n=2),
            )

    nc.compile()
    return nc


def kernel(x, mask, Wq, Wkv, Wout, b_out):
    global _last_results
    from concourse.bass_utils import run_bass_kernel_spmd

    bf = ml_dtypes.bfloat16
    x = np.asarray(x, dtype=np.float32)
    Wq = np.asarray(Wq, dtype=np.float32)
    Wkv = np.asarray(Wkv, dtype=np.float32)
    Wout = np.asarray(Wout, dtype=np.float32)
    b_out = np.asarray(b_out, dtype=np.float32)

    if "nc" not in _cached:
        _cached["nc"] = _build_program()
    nc = _cached["nc"]

    jj, ii = np.mgrid[0:128, 0:128]
    # pexp[j, o+c] is masked (multiplied by 0) where j > c
    tri = (jj <= ii).astype(np.float32).astype(bf)

    xTs = [np.ascontiguousarray(x[b].T).astype(bf) for b in range(B)]

    in_maps = []
    for c in range(NCORES):
        b = c // 4
        h0 = HPC * (c % 4)
        in_maps.append(
            {
                "xb": xTs[b],
                "wq": np.ascontiguousarray(Wq[:, DH * h0 : DH * (h0 + HPC)]).astype(bf),
                "wk": np.ascontiguousarray(Wkv[:, DH * h0 : DH * (h0 + HPC)]).astype(bf),
                "wv": np.ascontiguousarray(Wkv[:, D + DH * h0 : D + DH * (h0 + HPC)]).astype(bf),
                "wo": np.ascontiguousarray(Wout[DH * h0 : DH * (h0 + HPC), :]).astype(bf),
                "tri": tri,
            }
        )

    res = run_bass_kernel_spmd(
        nc,
        in_maps,
        core_ids=list(range(NCORES)),
        trace=bool(int(os.environ.get("KERNEL_TRACE", "0"))),
    )
    _last_results = res
    parts = [r["outp"] for r in res.results]
    out = np.empty((B, N, D), dtype=np.float32)
    for b in range(B):
        acc = parts[4 * b].astype(np.float32).copy()
        for c in range(4 * b + 1, 4 * b + 4):
            acc += parts[c]
        out[b] = acc + b_out[None, :]
    return out


# revision 25
# speedup vs baseline: 1.0967x; 1.0967x over previous
"""Causal multi-head attention kernel for 8 trn2 NeuronCores.

Problem: x[2,2048,1024], 16 heads of dim 64, causal softmax(q k^T / sqrt(1024)) v,
then output projection. Sharding: data-parallel over batch (4 cores per batch),
tensor-parallel over heads (4 heads per core). Each core produces a partial
output (its heads' contribution through Wout); the host sums the 4 partials per
batch and adds b_out.

Per-core device program (SPMD):
  - x arrives pre-transposed (host) as xT [d, n] in bf16; all weights bf16.
  - Projections: qT/kT [dh on partitions, n free] per head-PAIR (head A on
    partitions 0..63, head B on 64..127), v natural [n on partitions] with a
    shared ones column-block per (nb, pair): [dataA(64) | ones(64) | dataB(64)]
    so each head's AV matmul lhsT is a 128-col window whose ones half makes the
    matmul also produce softmax row-sums (A: sums on partitions 64..127,
    B: sums on partitions 0..63).
  - Attention in (cp = 512-wide i-chunk, pair, jb = 128-wide j-block) steps:
    the two heads' S^T = kT^T.qT matmuls have K=64 and run CONCURRENTLY on the
    PE via row-group tiling (head A rows 0-63, head B rows 64-127), writing the
    two halves of a pair-packed pS [128, 1024] (2 PSUM banks). The causal mask
    of the diagonal 128x128 block is applied ON THE PE as an extra accumulation
    matmul (lhsT = strictly-upper -30000 tile, rhs = identity), so the
    QK->exp->AV chain never crosses to the DVE. One ACT instruction computes
    exp for both heads through a 2-range strided AP. AV accumulates per-head
    O^T (+row sums) in PSUM over jb, lagged DELAY steps behind the exp so the
    PE never stalls on ACT. Block-causality skips all j>i blocks.
  - Normalization reads O^T straight from PSUM: reciprocal_approx_fast on the
    row-sum half + one tensor_mul -> OT in bf16 (no full-rate DVE reciprocal,
    no intermediate copy).
  - Output projection (contraction over the 4 heads = 2 pair-accumulated
    matmuls per [128n, 512d] tile) and the q/k/v projections stream through
    the attention steps as gated fillers to keep the PE dense (HAM warm).
"""

import os

import numpy as np
import ml_dtypes

B, N, D, H = 2, 2048, 1024, 16
DH = D // H  # 64
SCALE = float(D) ** -0.5
NCORES = 8
HPC = 4  # heads per core
NP = 2  # head pairs per core
IC = 512  # i-chunk width
NB = N // 128  # 16 j blocks
NCP = N // IC  # 4 i-chunks
KT = D // 128  # 8 contraction tiles
VW = 256  # v cols per (nb, pair): ones(64) | dataA(64) | ones(64) | dataB(64)
# sums land on pO partitions 0..63 for BOTH heads (reciprocal_approx_fast only
# works at base_partition 0), data on partitions 64..127
DELAY = 2
MASKV = -30000.0

_cached = {}
_last_results = None


def _build_program():
    import concourse.bacc as bacc
    import concourse.mybir as mybir
    import concourse.tile as tile

    f32 = mybir.dt.float32
    bf16 = mybir.dt.bfloat16
    EXP = mybir.ActivationFunctionType.Exp

    nc = bacc.Bacc()

    xb = nc.dram_tensor("xb", [D, N], bf16, kind="ExternalInput")  # x^T
    wq = nc.dram_tensor("wq", [D, HPC * DH], bf16, kind="ExternalInput")
    wk = nc.dram_tensor("wk", [D, HPC * DH], bf16, kind="ExternalInput")
    wv = nc.dram_tensor("wv", [D, HPC * DH], bf16, kind="ExternalInput")
    wo = nc.dram_tensor("wo", [HPC * DH, D], bf16, kind="ExternalInput")
    tri = nc.dram_tensor("tri", [128, 128], bf16, kind="ExternalInput")
    outp = nc.dram_tensor("outp", [N, D], f32, kind="ExternalOutput")

    with tile.TileContext(nc) as tc:
        with (
            tc.tile_pool(name="const", bufs=1) as const_pool,
            tc.tile_pool(name="big", bufs=1) as big_pool,
            tc.tile_pool(name="pS", bufs=2, space="PSUM") as pS_pool,
            tc.tile_pool(name="pO", bufs=2, space="PSUM") as pO_pool,
            tc.tile_pool(name="pj", bufs=2, space="PSUM") as pj_pool,
            tc.tile_pool(name="att", bufs=4) as att_pool,
            tc.tile_pool(name="rec", bufs=4) as rec_pool,
            tc.tile_pool(name="osb", bufs=3) as osb_pool,
        ):
            # Dummy exp early: pulls the ~2.7us ACT table load off the
            # critical path (overlaps the initial DMA).
            warm = const_pool.tile([1, 8], f32, name="warm", tag="warm")
            nc.vector.memset(warm, 0.0)
            nc.scalar.activation(out=warm, in_=warm, func=EXP, scale=1.0)
            # ~3.5us of dummy matmuls while the input DMA runs: trips the HAM
            # activity window so the real stream starts at 2.4GHz, not 1.2
            wa = const_pool.tile([128, 512], bf16, name="wa", tag="wa")
            nc.vector.memset(wa, 0.0)
            for _ in range(9):
                pwarm = pj_pool.tile([128, 512], f32, name="pwarm", tag="pj")
                nc.tensor.matmul(pwarm, lhsT=wa[:, 0:128], rhs=wa, start=True, stop=True)

            # one DMA instruction per weight tensor: d-tile r lands at free-dim
            # group r of a single [128, KT*256] tile (issue cost on the sync
            # queue is ~0.6us per DMA instruction, so batching matters)
            wqa = const_pool.tile([128, KT * 256], bf16, name="wqa", tag="wqa")
            wka = const_pool.tile([128, KT * 256], bf16, name="wka", tag="wka")
            wva = const_pool.tile([128, KT * 256], bf16, name="wva", tag="wva")
            woa = const_pool.tile([128, NP * D], bf16, name="woa", tag="woa")
            nc.sync.dma_start(
                out=wva, in_=wv[:, :].rearrange("(r p) c -> p r c", r=KT)
            )
            xTall = big_pool.tile([128, KT * N], bf16, name="xTall", tag="xTall")
            xT = [xTall[:, N * r : N * (r + 1)] for r in range(KT)]
            xT4 = xTall.rearrange("p (r c) -> p r c", r=KT)
            # first column-quarter of x^T lands first so projections start early
            nc.sync.dma_start(
                out=xT4[:, :, 0:512],
                in_=xb[:, 0:512].rearrange("(r p) c -> p r c", r=KT),
            )
            nc.sync.dma_start(
                out=xT4[:, :, 512:1024],
                in_=xb[:, 512:1024].rearrange("(r p) c -> p r c", r=KT),
            )
            nc.sync.dma_start(
                out=wqa, in_=wq[:, :].rearrange("(r p) c -> p r c", r=KT)
            )
            nc.sync.dma_start(
                out=wka, in_=wk[:, :].rearrange("(r p) c -> p r c", r=KT)
            )
            tri_sb = const_pool.tile([128, 128], bf16, name="tri_sb", tag="tri_sb")
            nc.sync.dma_start(out=tri_sb, in_=tri[:, :])
            nc.sync.dma_start(
                out=xT4[:, :, 1024:2048],
                in_=xb[:, 1024:2048].rearrange("(r p) c -> p r c", r=KT),
            )
            nc.sync.dma_start(
                out=woa, in_=wo[:, :].rearrange("(p q) c -> q p c", p=NP)
            )
            wq_sb = [wqa[:, 256 * r : 256 * (r + 1)] for r in range(KT)]
            wk_sb = [wka[:, 256 * r : 256 * (r + 1)] for r in range(KT)]
            wv_sb = [wva[:, 256 * r : 256 * (r + 1)] for r in range(KT)]
            wo_sb = [woa[:, D * p : D * (p + 1)] for p in range(NP)]

            qT, kT_ = [], []
            for p in range(NP):
                qT.append(big_pool.tile([128, N], bf16, name=f"qT{p}", tag=f"qT{p}"))
                kT_.append(big_pool.tile([128, N], bf16, name=f"kT{p}", tag=f"kT{p}"))
            v_all = big_pool.tile([128, NB * NP * VW], bf16, name="v_all", tag="v_all")
            # ones for the row-sum trick; data cols overwritten by vproj copies
            nc.vector.memset(v_all, 1.0)
            OT = []
            for p in range(NP):
                OT.append(big_pool.tile([128, N], bf16, name=f"OT{p}", tag=f"OT{p}"))

            va8 = v_all.rearrange("p (n g c) -> p n g c", n=NB, g=8)

            def vproj_stream(nbs):
                for nb in nbs:
                    pv = pj_pool.tile([128, HPC * DH], f32, name="pv", tag="pj")
                    for r in range(KT):
                        nc.tensor.matmul(
                            pv,
                            lhsT=xT[r][:, 128 * nb : 128 * (nb + 1)],
                            rhs=wv_sb[r],
                            start=(r == 0),
                            stop=(r == KT - 1),
                        )
                    # head h data -> 64-col group 2h+1 (odd groups; evens stay ones)
                    pv4 = pv.rearrange("p (h c) -> p h c", h=HPC)
                    nc.vector.tensor_copy(out=va8[:, nb, 1::2, :], in_=pv4)
                    yield

            def qkproj_stream(p, cs):
                for c in cs:
                    sl = slice(IC * c, IC * (c + 1))
                    for w_sb, dst in ((wq_sb, qT[p]), (wk_sb, kT_[p])):
                        pq = pj_pool.tile([128, IC], f32, name="pq", tag="pj")
                        for r in range(KT):
                            nc.tensor.matmul(
                                pq,
                                lhsT=w_sb[r][:, 128 * p : 128 * (p + 1)],
                                rhs=xT[r][:, sl],
                                start=(r == 0),
                                stop=(r == KT - 1),
                            )
                        nc.vector.tensor_copy(out=dst[:, sl], in_=pq)
                        yield

            def outproj_stream(nbs):
                for nb in nbs:
                    nsl = slice(128 * nb, 128 * (nb + 1))
                    for s in range(2):
                        po = pj_pool.tile([128, 512], f32, name="po", tag="pj")
                        for p in range(NP):
                            nc.tensor.matmul(
                                po,
                                lhsT=OT[p][:, nsl],
                                rhs=wo_sb[p][:, 512 * s : 512 * (s + 1)],
                                start=(p == 0),
                                stop=(p == NP - 1),
                            )
                        ob = osb_pool.tile([128, 512], f32, name="ob", tag="osb")
                        # DVE only: a scalar.copy's sem-wait would head-of-line
                        # block the exp stream on the Scalar queue
                        nc.vector.tensor_copy(out=ob, in_=po)
                        nc.gpsimd.dma_start(out=outp[nsl, 512 * s : 512 * (s + 1)], in_=ob)
                        yield

            pend = []

            def drain(n):
                while len(pend) > n:
                    pend.pop(0)()

            def attention_stream():
                for cp in range(NCP):
                    for p in range(NP):
                        pO_A = pO_pool.tile([128, IC], f32, name=f"pOA{cp}{p}", tag="pO")
                        pO_B = pO_pool.tile([128, IC], f32, name=f"pOB{cp}{p}", tag="pO")
                        jmax = 4 * cp + 4
                        for jb in range(jmax):
                            o = max(0, 128 * jb - IC * cp)
                            jsl = slice(128 * jb, 128 * (jb + 1))
                            isl = slice(IC * cp + o, IC * (cp + 1))
                            pS = pS_pool.tile([128, 2 * IC], f32, name="pS", tag="pS")
                            pexp = att_pool.tile([128, 2 * IC], bf16, name="pexp", tag="pexp")
                            # S^T pair: K=64 each, concurrent via row groups
                            nc.tensor.matmul(
                                pS[:, o:IC],
                                lhsT=kT_[p][0:64, jsl],
                                rhs=qT[p][0:64, isl],
                                start=True,
                                stop=True,
                            )
                            nc.tensor.matmul(
                                pS[:, IC + o : 2 * IC],
                                lhsT=kT_[p][64:128, jsl],
                                rhs=qT[p][64:128, isl],
                                start=True,
                                stop=True,
                            )
                            # one exp for both heads: [128, 2, IC-o] strided AP
                            src = pS.rearrange("p (h w) -> p h w", h=2)[:, :, o:]
                            dst = pexp.rearrange("p (h w) -> p h w", h=2)[:, :, o:]
                            nc.scalar.activation(out=dst, in_=src, func=EXP, scale=SCALE)
                            if 128 * jb >= IC * cp:  # diagonal block: 0/1 mask
                                for half in range(2):
                                    hb = IC * half
                                    nc.vector.tensor_mul(
                                        pexp[:, hb + o : hb + o + 128],
                                        pexp[:, hb + o : hb + o + 128],
                                        tri_sb,
                                    )

                            def av_unit(p=p, jb=jb, o=o, jmax=jmax, pO_A=pO_A, pO_B=pO_B, pexp=pexp):
                                vo = 2 * VW * jb + VW * p
                                nc.tensor.matmul(
                                    pO_A[:, o:IC],
                                    lhsT=v_all[:, vo : vo + 128],
                                    rhs=pexp[:, o:IC],
                                    start=(jb == 0),
                                    stop=(jb == jmax - 1),
                                    skip_group_check=True,
                                )
                                nc.tensor.matmul(
                                    pO_B[:, o:IC],
                                    lhsT=v_all[:, vo + 128 : vo + 256],
                                    rhs=pexp[:, IC + o : 2 * IC],
                                    start=(jb == 0),
                                    stop=(jb == jmax - 1),
                                    skip_group_check=True,
                                )

                            pend.append(av_unit)
                            drain(DELAY)
                            yield

                        # normalize straight from PSUM; OT written in bf16
                        csl = slice(IC * cp, IC * (cp + 1))
                        rec_A = rec_pool.tile([64, IC], f32, name="recA", tag="rec")
                        rec_B = rec_pool.tile([64, IC], f32, name="recB", tag="rec")

                        def recip_a(pO_A=pO_A, rec_A=rec_A):
                            nc.vector.reciprocal_approx_fast(out=rec_A, in_=pO_A[0:64, :])

                        def mul_a(pO_A=pO_A, rec_A=rec_A, p=p, csl=csl):
                            nc.vector.tensor_mul(OT[p][0:64, csl], pO_A[64:128, :], rec_A)

                        def recip_b(pO_B=pO_B, rec_B=rec_B):
                            nc.vector.reciprocal_approx_fast(out=rec_B, in_=pO_B[0:64, :])

                        def mul_b(pO_B=pO_B, rec_B=rec_B, p=p, csl=csl):
                            nc.vector.tensor_mul(OT[p][64:128, csl], pO_B[64:128, :], rec_B)

                        pend.append(recip_a)
                        pend.append(mul_a)
                        pend.append(recip_b)
                        pend.append(mul_b)

            # ---- prologue: everything attention (cp0) needs ----
            for _ in vproj_stream(range(0, 4)):
                pass
            for _ in qkproj_stream(0, [0]):
                pass
            for _ in qkproj_stream(1, [0]):
                pass

            # ---- gated fillers pulled between attention steps ----
            fillers = [
                (0, qkproj_stream(0, [1])),
                (0, qkproj_stream(1, [1])),
                (0, vproj_stream(range(4, 8))),
                (8, qkproj_stream(0, [2])),
                (8, qkproj_stream(1, [2])),
                (8, vproj_stream(range(8, 12))),
                (12, outproj_stream(range(0, 4))),
                (24, qkproj_stream(0, [3])),
                (24, qkproj_stream(1, [3])),
                (24, vproj_stream(range(12, 16))),
                (28, outproj_stream(range(4, 8))),
                (52, outproj_stream(range(8, 12))),
            ]

            def pull_filler(step):
                for i, (gate, gen) in enumerate(fillers):
                    if step < gate:
                        continue
                    if next(gen, "end") == "end":
                        fillers.pop(i)
                        continue
                    return True
                return False

            att = attention_stream()
            for step, _ in enumerate(att):
                pull_filler(step)
                if step >= 48:
                    pull_filler(step)
            drain(0)
            for _, gen in fillers:
                for _ in gen:
                    pass
            # pair 1's cp3 normalize is only issued by drain(0) above, so the
            # last output-projection blocks must stay in the epilogue. Stage
            # them in one wide SBUF tile and ship with a single DMA (per-DMA
            # issue costs ~0.6us on the queue, the dominant tail cost).
            obig = big_pool.tile([128, 4 * D], f32, name="obig", tag="obig")
            for i, nb in enumerate(range(12, 16)):
                nsl = slice(128 * nb, 128 * (nb + 1))
                for s in range(2):
                    po = pj_pool.tile([128, 512], f32, name="po", tag="pj")
                    for p in range(NP):
                        nc.tensor.matmul(
                            po,
                            lhsT=OT[p][:, nsl],
                            rhs=wo_sb[p][:, 512 * s : 512 * (s + 1)],
                            start=(p == 0),
                            stop=(p == NP - 1),
                        )
                    dst = obig[:, D * i + 512 * s : D * i + 512 * (s + 1)]
                    if s == 0:
                        nc.vector.tensor_copy(out=dst, in_=po)
                    else:
                        nc.scalar.copy(out=dst, in_=po)
                if nb == 13:
                    nc.sync.dma_start(
                        out=outp[1536:1792, :].rearrange("(n p) c -> p n c", n=2),
                        in_=obig[:, 0 : 2 * D].rearrange("p (n c) -> p n c", n=2),
                    )
            nc.sync.dma_start(
                out=outp[1792:2048, :].rearrange("(n p) c -> p n c", n=2),
                in_=obig[:, 2 * D : 4 * D].rearrange("p (n c) -> p n c", n=2),
            )

    nc.compile()
    return nc


def kernel(x, mask, Wq, Wkv, Wout, b_out):
    global _last_results
    from concourse.bass_utils import run_bass_kernel_spmd

    bf = ml_dtypes.bfloat16
    x = np.asarray(x, dtype=np.float32)
    Wq = np.asarray(Wq, dtype=np.float32)
    Wkv = np.asarray(Wkv, dtype=np.float32)
    Wout = np.asarray(Wout, dtype=np.float32)
    b_out = np.asarray(b_out, dtype=np.float32)

    if "nc" not in _cached:
        _cached["nc"] = _build_program()
    nc = _cached["nc"]

    jj, ii = np.mgrid[0:128, 0:128]
    # pexp[j, o+c] is masked (multiplied by 0) where j > c
    tri = (jj <= ii).astype(np.float32).astype(bf)

    xTs = [np.ascontiguousarray(x[b].T).astype(bf) for b in range(B)]

    in_maps = []
    for c in range(NCORES):
        b = c // 4
        h0 = HPC * (c % 4)
        in_maps.append(
            {
                "xb": xTs[b],
                "wq": np.ascontiguousarray(Wq[:, DH * h0 : DH * (h0 + HPC)]).astype(bf),
                "wk": np.ascontiguousarray(Wkv[:, DH * h0 : DH * (h0 + HPC)]).astype(bf),
                "wv": np.ascontiguousarray(Wkv[:, D + DH * h0 : D + DH * (h0 + HPC)]).astype(bf),
                "wo": np.ascontiguousarray(Wout[DH * h0 : DH * (h0 + HPC), :]).astype(bf),
                "tri": tri,
            }
        )

    res = run_bass_kernel_spmd(
        nc,
        in_maps,
        core_ids=list(range(NCORES)),
        trace=bool(int(os.environ.get("KERNEL_TRACE", "0"))),
    )
    _last_results = res
    parts = [r["outp"] for r in res.results]
    out = np.empty((B, N, D), dtype=np.float32)
    for b in range(B):
        acc = parts[4 * b].astype(np.float32).copy()
        for c in range(4 * b + 1, 4 * b + 4):
            acc += parts[c]
        out[b] = acc + b_out[None, :]
    return out


# revision 26
# speedup vs baseline: 1.1135x; 1.0153x over previous
"""Causal multi-head attention kernel for 8 trn2 NeuronCores.

Problem: x[2,2048,1024], 16 heads of dim 64, causal softmax(q k^T / sqrt(1024)) v,
then output projection. Sharding: data-parallel over batch (4 cores per batch),
tensor-parallel over heads (4 heads per core). Each core produces a partial
output (its heads' contribution through Wout); the host sums the 4 partials per
batch and adds b_out.

Per-core device program (SPMD):
  - x arrives pre-transposed (host) as xT [d, n] in bf16; all weights bf16.
  - Projections: qT/kT [dh on partitions, n free] per head-PAIR (head A on
    partitions 0..63, head B on 64..127), v natural [n on partitions] with a
    shared ones column-block per (nb, pair): [dataA(64) | ones(64) | dataB(64)]
    so each head's AV matmul lhsT is a 128-col window whose ones half makes the
    matmul also produce softmax row-sums (A: sums on partitions 64..127,
    B: sums on partitions 0..63).
  - Attention in (cp = 512-wide i-chunk, pair, jb = 128-wide j-block) steps:
    the two heads' S^T = kT^T.qT matmuls have K=64 and run CONCURRENTLY on the
    PE via row-group tiling (head A rows 0-63, head B rows 64-127), writing the
    two halves of a pair-packed pS [128, 1024] (2 PSUM banks). The causal mask
    of the diagonal 128x128 block is applied ON THE PE as an extra accumulation
    matmul (lhsT = strictly-upper -30000 tile, rhs = identity), so the
    QK->exp->AV chain never crosses to the DVE. One ACT instruction computes
    exp for both heads through a 2-range strided AP. AV accumulates per-head
    O^T (+row sums) in PSUM over jb, lagged DELAY steps behind the exp so the
    PE never stalls on ACT. Block-causality skips all j>i blocks.
  - Normalization reads O^T straight from PSUM: reciprocal_approx_fast on the
    row-sum half + one tensor_mul -> OT in bf16 (no full-rate DVE reciprocal,
    no intermediate copy).
  - Output projection (contraction over the 4 heads = 2 pair-accumulated
    matmuls per [128n, 512d] tile) and the q/k/v projections stream through
    the attention steps as gated fillers to keep the PE dense (HAM warm).
"""

import os

import numpy as np
import ml_dtypes

B, N, D, H = 2, 2048, 1024, 16
DH = D // H  # 64
SCALE = float(D) ** -0.5
NCORES = 8
HPC = 4  # heads per core
NP = 2  # head pairs per core
IC = 512  # i-chunk width
NB = N // 128  # 16 j blocks
NCP = N // IC  # 4 i-chunks
KT = D // 128  # 8 contraction tiles
VW = 256  # v cols per (nb, pair): ones(64) | dataA(64) | ones(64) | dataB(64)
# sums land on pO partitions 0..63 for BOTH heads (reciprocal_approx_fast only
# works at base_partition 0), data on partitions 64..127
DELAY = 2
MASKV = -30000.0

_cached = {}
_last_results = None


def _build_program():
    import concourse.bacc as bacc
    import concourse.mybir as mybir
    import concourse.tile as tile

    f32 = mybir.dt.float32
    bf16 = mybir.dt.bfloat16
    EXP = mybir.ActivationFunctionType.Exp

    nc = bacc.Bacc()

    xb = nc.dram_tensor("xb", [D, N], bf16, kind="ExternalInput")  # x^T
    wq = nc.dram_tensor("wq", [D, HPC * DH], bf16, kind="ExternalInput")
    wk = nc.dram_tensor("wk", [D, HPC * DH], bf16, kind="ExternalInput")
    wv = nc.dram_tensor("wv", [D, HPC * DH], bf16, kind="ExternalInput")
    wo = nc.dram_tensor("wo", [HPC * DH, D], bf16, kind="ExternalInput")
    tri = nc.dram_tensor("tri", [128, 128], bf16, kind="ExternalInput")
    outp = nc.dram_tensor("outp", [N, D], bf16, kind="ExternalOutput")

    with tile.TileContext(nc) as tc:
        with (
            tc.tile_pool(name="const", bufs=1) as const_pool,
            tc.tile_pool(name="big", bufs=1) as big_pool,
            tc.tile_pool(name="pS", bufs=2, space="PSUM") as pS_pool,
            tc.tile_pool(name="pO", bufs=2, space="PSUM") as pO_pool,
            tc.tile_pool(name="pj", bufs=2, space="PSUM") as pj_pool,
            tc.tile_pool(name="att", bufs=4) as att_pool,
            tc.tile_pool(name="rec", bufs=4) as rec_pool,
            tc.tile_pool(name="osb", bufs=3) as osb_pool,
        ):
            # Dummy exp early: pulls the ~2.7us ACT table load off the
            # critical path (overlaps the initial DMA).
            warm = const_pool.tile([1, 8], f32, name="warm", tag="warm")
            nc.vector.memset(warm, 0.0)
            nc.scalar.activation(out=warm, in_=warm, func=EXP, scale=1.0)
            # ~3.5us of dummy matmuls while the input DMA runs: trips the HAM
            # activity window so the real stream starts at 2.4GHz, not 1.2
            wa = const_pool.tile([128, 512], bf16, name="wa", tag="wa")
            nc.vector.memset(wa, 0.0)
            for _ in range(9):
                pwarm = pj_pool.tile([128, 512], f32, name="pwarm", tag="pj")
                nc.tensor.matmul(pwarm, lhsT=wa[:, 0:128], rhs=wa, start=True, stop=True)

            # one DMA instruction per weight tensor: d-tile r lands at free-dim
            # group r of a single [128, KT*256] tile (issue cost on the sync
            # queue is ~0.6us per DMA instruction, so batching matters)
            wqa = const_pool.tile([128, KT * 256], bf16, name="wqa", tag="wqa")
            wka = const_pool.tile([128, KT * 256], bf16, name="wka", tag="wka")
            wva = const_pool.tile([128, KT * 256], bf16, name="wva", tag="wva")
            woa = const_pool.tile([128, NP * D], bf16, name="woa", tag="woa")
            nc.sync.dma_start(
                out=wva, in_=wv[:, :].rearrange("(r p) c -> p r c", r=KT)
            )
            xTall = big_pool.tile([128, KT * N], bf16, name="xTall", tag="xTall")
            xT = [xTall[:, N * r : N * (r + 1)] for r in range(KT)]
            xT4 = xTall.rearrange("p (r c) -> p r c", r=KT)
            # first column-quarter of x^T lands first so projections start early
            nc.sync.dma_start(
                out=xT4[:, :, 0:512],
                in_=xb[:, 0:512].rearrange("(r p) c -> p r c", r=KT),
            )
            nc.sync.dma_start(
                out=xT4[:, :, 512:1024],
                in_=xb[:, 512:1024].rearrange("(r p) c -> p r c", r=KT),
            )
            nc.sync.dma_start(
                out=wqa, in_=wq[:, :].rearrange("(r p) c -> p r c", r=KT)
            )
            nc.sync.dma_start(
                out=wka, in_=wk[:, :].rearrange("(r p) c -> p r c", r=KT)
            )
            tri_sb = const_pool.tile([128, 128], bf16, name="tri_sb", tag="tri_sb")
            nc.sync.dma_start(out=tri_sb, in_=tri[:, :])
            nc.sync.dma_start(
                out=xT4[:, :, 1024:2048],
                in_=xb[:, 1024:2048].rearrange("(r p) c -> p r c", r=KT),
            )
            nc.sync.dma_start(
                out=woa, in_=wo[:, :].rearrange("(p q) c -> q p c", p=NP)
            )
            wq_sb = [wqa[:, 256 * r : 256 * (r + 1)] for r in range(KT)]
            wk_sb = [wka[:, 256 * r : 256 * (r + 1)] for r in range(KT)]
            wv_sb = [wva[:, 256 * r : 256 * (r + 1)] for r in range(KT)]
            wo_sb = [woa[:, D * p : D * (p + 1)] for p in range(NP)]

            qT, kT_ = [], []
            for p in range(NP):
                qT.append(big_pool.tile([128, N], bf16, name=f"qT{p}", tag=f"qT{p}"))
                kT_.append(big_pool.tile([128, N], bf16, name=f"kT{p}", tag=f"kT{p}"))
            v_all = big_pool.tile([128, NB * NP * VW], bf16, name="v_all", tag="v_all")
            # ones for the row-sum trick; data cols overwritten by vproj copies
            nc.vector.memset(v_all, 1.0)
            OT = []
            for p in range(NP):
                OT.append(big_pool.tile([128, N], bf16, name=f"OT{p}", tag=f"OT{p}"))

            va8 = v_all.rearrange("p (n g c) -> p n g c", n=NB, g=8)

            def vproj_stream(nbs):
                for nb in nbs:
                    pv = pj_pool.tile([128, HPC * DH], f32, name="pv", tag="pj")
                    for r in range(KT):
                        nc.tensor.matmul(
                            pv,
                            lhsT=xT[r][:, 128 * nb : 128 * (nb + 1)],
                            rhs=wv_sb[r],
                            start=(r == 0),
                            stop=(r == KT - 1),
                        )
                    # head h data -> 64-col group 2h+1 (odd groups; evens stay ones)
                    pv4 = pv.rearrange("p (h c) -> p h c", h=HPC)
                    nc.vector.tensor_copy(out=va8[:, nb, 1::2, :], in_=pv4)
                    yield

            def qkproj_stream(p, cs):
                for c in cs:
                    sl = slice(IC * c, IC * (c + 1))
                    for w_sb, dst in ((wq_sb, qT[p]), (wk_sb, kT_[p])):
                        pq = pj_pool.tile([128, IC], f32, name="pq", tag="pj")
                        for r in range(KT):
                            nc.tensor.matmul(
                                pq,
                                lhsT=w_sb[r][:, 128 * p : 128 * (p + 1)],
                                rhs=xT[r][:, sl],
                                start=(r == 0),
                                stop=(r == KT - 1),
                            )
                        nc.vector.tensor_copy(out=dst[:, sl], in_=pq)
                        yield

            def outproj_stream(nbs):
                for nb in nbs:
                    nsl = slice(128 * nb, 128 * (nb + 1))
                    for s in range(2):
                        po = pj_pool.tile([128, 512], f32, name="po", tag="pj")
                        for p in range(NP):
                            nc.tensor.matmul(
                                po,
                                lhsT=OT[p][:, nsl],
                                rhs=wo_sb[p][:, 512 * s : 512 * (s + 1)],
                                start=(p == 0),
                                stop=(p == NP - 1),
                            )
                        ob = osb_pool.tile([128, 512], bf16, name="ob", tag="osb")
                        # DVE only: a scalar.copy's sem-wait would head-of-line
                        # block the exp stream on the Scalar queue
                        nc.vector.tensor_copy(out=ob, in_=po)
                        nc.gpsimd.dma_start(out=outp[nsl, 512 * s : 512 * (s + 1)], in_=ob)
                        yield

            pend = []

            def drain(n):
                while len(pend) > n:
                    pend.pop(0)()

            def attention_stream():
                for cp in range(NCP):
                    for p in range(NP):
                        pO_A = pO_pool.tile([128, IC], f32, name=f"pOA{cp}{p}", tag="pO")
                        pO_B = pO_pool.tile([128, IC], f32, name=f"pOB{cp}{p}", tag="pO")
                        jmax = 4 * cp + 4
                        for jb in range(jmax):
                            o = max(0, 128 * jb - IC * cp)
                            jsl = slice(128 * jb, 128 * (jb + 1))
                            isl = slice(IC * cp + o, IC * (cp + 1))
                            pS = pS_pool.tile([128, 2 * IC], f32, name="pS", tag="pS")
                            pexp = att_pool.tile([128, 2 * IC], bf16, name="pexp", tag="pexp")
                            # S^T pair: K=64 each, concurrent via row groups
                            nc.tensor.matmul(
                                pS[:, o:IC],
                                lhsT=kT_[p][0:64, jsl],
                                rhs=qT[p][0:64, isl],
                                start=True,
                                stop=True,
                            )
                            nc.tensor.matmul(
                                pS[:, IC + o : 2 * IC],
                                lhsT=kT_[p][64:128, jsl],
                                rhs=qT[p][64:128, isl],
                                start=True,
                                stop=True,
                            )
                            # one exp for both heads: [128, 2, IC-o] strided AP
                            src = pS.rearrange("p (h w) -> p h w", h=2)[:, :, o:]
                            dst = pexp.rearrange("p (h w) -> p h w", h=2)[:, :, o:]
                            nc.scalar.activation(out=dst, in_=src, func=EXP, scale=SCALE)
                            if 128 * jb >= IC * cp:  # diagonal block: 0/1 mask
                                for half in range(2):
                                    hb = IC * half
                                    nc.vector.tensor_mul(
                                        pexp[:, hb + o : hb + o + 128],
                                        pexp[:, hb + o : hb + o + 128],
                                        tri_sb,
                                    )

                            def av_unit(p=p, jb=jb, o=o, jmax=jmax, pO_A=pO_A, pO_B=pO_B, pexp=pexp):
                                vo = 2 * VW * jb + VW * p
                                nc.tensor.matmul(
                                    pO_A[:, o:IC],
                                    lhsT=v_all[:, vo : vo + 128],
                                    rhs=pexp[:, o:IC],
                                    start=(jb == 0),
                                    stop=(jb == jmax - 1),
                                    skip_group_check=True,
                                )
                                nc.tensor.matmul(
                                    pO_B[:, o:IC],
                                    lhsT=v_all[:, vo + 128 : vo + 256],
                                    rhs=pexp[:, IC + o : 2 * IC],
                                    start=(jb == 0),
                                    stop=(jb == jmax - 1),
                                    skip_group_check=True,
                                )

                            pend.append(av_unit)
                            drain(DELAY)
                            yield

                        # normalize straight from PSUM; OT written in bf16
                        csl = slice(IC * cp, IC * (cp + 1))
                        rec_A = rec_pool.tile([64, IC], f32, name="recA", tag="rec")
                        rec_B = rec_pool.tile([64, IC], f32, name="recB", tag="rec")

                        def recip_a(pO_A=pO_A, rec_A=rec_A):
                            nc.vector.reciprocal_approx_fast(out=rec_A, in_=pO_A[0:64, :])

                        def mul_a(pO_A=pO_A, rec_A=rec_A, p=p, csl=csl):
                            nc.vector.tensor_mul(OT[p][0:64, csl], pO_A[64:128, :], rec_A)

                        def recip_b(pO_B=pO_B, rec_B=rec_B):
                            nc.vector.reciprocal_approx_fast(out=rec_B, in_=pO_B[0:64, :])

                        def mul_b(pO_B=pO_B, rec_B=rec_B, p=p, csl=csl):
                            nc.vector.tensor_mul(OT[p][64:128, csl], pO_B[64:128, :], rec_B)

                        pend.append(recip_a)
                        pend.append(mul_a)
                        pend.append(recip_b)
                        pend.append(mul_b)

            # ---- prologue: everything attention (cp0) needs ----
            for _ in vproj_stream(range(0, 4)):
                pass
            for _ in qkproj_stream(0, [0]):
                pass
            for _ in qkproj_stream(1, [0]):
                pass

            # ---- gated fillers pulled between attention steps ----
            fillers = [
                (0, qkproj_stream(0, [1])),
                (0, qkproj_stream(1, [1])),
                (0, vproj_stream(range(4, 8))),
                (8, qkproj_stream(0, [2])),
                (8, qkproj_stream(1, [2])),
                (8, vproj_stream(range(8, 12))),
                (12, outproj_stream(range(0, 4))),
                (24, qkproj_stream(0, [3])),
                (24, qkproj_stream(1, [3])),
                (24, vproj_stream(range(12, 16))),
                (28, outproj_stream(range(4, 8))),
                (52, outproj_stream(range(8, 12))),
            ]

            def pull_filler(step):
                for i, (gate, gen) in enumerate(fillers):
                    if step < gate:
                        continue
                    if next(gen, "end") == "end":
                        fillers.pop(i)
                        continue
                    return True
                return False

            att = attention_stream()
            for step, _ in enumerate(att):
                pull_filler(step)
                if step >= 48:
                    pull_filler(step)
            drain(0)
            for _, gen in fillers:
                for _ in gen:
                    pass
            # pair 1's cp3 normalize is only issued by drain(0) above, so the
            # last output-projection blocks must stay in the epilogue. Stage
            # them in one wide SBUF tile and ship with a single DMA (per-DMA
            # issue costs ~0.6us on the queue, the dominant tail cost).
            obig = big_pool.tile([128, 4 * D], bf16, name="obig", tag="obig")
            for i, nb in enumerate(range(12, 16)):
                nsl = slice(128 * nb, 128 * (nb + 1))
                for s in range(2):
                    po = pj_pool.tile([128, 512], f32, name="po", tag="pj")
                    for p in range(NP):
                        nc.tensor.matmul(
                            po,
                            lhsT=OT[p][:, nsl],
                            rhs=wo_sb[p][:, 512 * s : 512 * (s + 1)],
                            start=(p == 0),
                            stop=(p == NP - 1),
                        )
                    dst = obig[:, D * i + 512 * s : D * i + 512 * (s + 1)]
                    if s == 0:
                        nc.vector.tensor_copy(out=dst, in_=po)
                    else:
                        nc.scalar.copy(out=dst, in_=po)
                if nb == 13:
                    nc.sync.dma_start(
                        out=outp[1536:1792, :].rearrange("(n p) c -> p n c", n=2),
                        in_=obig[:, 0 : 2 * D].rearrange("p (n c) -> p n c", n=2),
                    )
            nc.sync.dma_start(
                out=outp[1792:2048, :].rearrange("(n p) c -> p n c", n=2),
                in_=obig[:, 2 * D : 4 * D].rearrange("p (n c) -> p n c", n=2),
            )

    nc.compile()
    return nc


def kernel(x, mask, Wq, Wkv, Wout, b_out):
    global _last_results
    from concourse.bass_utils import run_bass_kernel_spmd

    bf = ml_dtypes.bfloat16
    x = np.asarray(x, dtype=np.float32)
    Wq = np.asarray(Wq, dtype=np.float32)
    Wkv = np.asarray(Wkv, dtype=np.float32)
    Wout = np.asarray(Wout, dtype=np.float32)
    b_out = np.asarray(b_out, dtype=np.float32)

    if "nc" not in _cached:
        _cached["nc"] = _build_program()
    nc = _cached["nc"]

    jj, ii = np.mgrid[0:128, 0:128]
    # pexp[j, o+c] is masked (multiplied by 0) where j > c
    tri = (jj <= ii).astype(np.float32).astype(bf)

    xTs = [np.ascontiguousarray(x[b].T).astype(bf) for b in range(B)]

    in_maps = []
    for c in range(NCORES):
        b = c // 4
        h0 = HPC * (c % 4)
        in_maps.append(
            {
                "xb": xTs[b],
                "wq": np.ascontiguousarray(Wq[:, DH * h0 : DH * (h0 + HPC)]).astype(bf),
                "wk": np.ascontiguousarray(Wkv[:, DH * h0 : DH * (h0 + HPC)]).astype(bf),
                "wv": np.ascontiguousarray(Wkv[:, D + DH * h0 : D + DH * (h0 + HPC)]).astype(bf),
                "wo": np.ascontiguousarray(Wout[DH * h0 : DH * (h0 + HPC), :]).astype(bf),
                "tri": tri,
            }
        )

    res = run_bass_kernel_spmd(
        nc,
        in_maps,
        core_ids=list(range(NCORES)),
        trace=bool(int(os.environ.get("KERNEL_TRACE", "0"))),
    )
    _last_results = res
    parts = [r["outp"] for r in res.results]
    out = np.empty((B, N, D), dtype=np.float32)
    for b in range(B):
        acc = parts[4 * b].astype(np.float32).copy()
        for c in range(4 * b + 1, 4 * b + 4):
            acc += parts[c]
        out[b] = acc + b_out[None, :]
    return out


# revision 28
# speedup vs baseline: 1.1149x; 1.0012x over previous
"""Causal multi-head attention kernel for 8 trn2 NeuronCores.

Problem: x[2,2048,1024], 16 heads of dim 64, causal softmax(q k^T / sqrt(1024)) v,
then output projection. Sharding: data-parallel over batch (4 cores per batch),
tensor-parallel over heads (4 heads per core). Each core produces a partial
output (its heads' contribution through Wout); the host sums the 4 partials per
batch and adds b_out.

Per-core device program (SPMD):
  - x arrives pre-transposed (host) as xT [d, n] in bf16; all weights bf16.
  - Projections: qT/kT [dh on partitions, n free] per head-PAIR (head A on
    partitions 0..63, head B on 64..127), v natural [n on partitions] with a
    shared ones column-block per (nb, pair): [dataA(64) | ones(64) | dataB(64)]
    so each head's AV matmul lhsT is a 128-col window whose ones half makes the
    matmul also produce softmax row-sums (A: sums on partitions 64..127,
    B: sums on partitions 0..63).
  - Attention in (cp = 512-wide i-chunk, pair, jb = 128-wide j-block) steps:
    the two heads' S^T = kT^T.qT matmuls have K=64 and run CONCURRENTLY on the
    PE via row-group tiling (head A rows 0-63, head B rows 64-127), writing the
    two halves of a pair-packed pS [128, 1024] (2 PSUM banks). The causal mask
    of the diagonal 128x128 block is applied ON THE PE as an extra accumulation
    matmul (lhsT = strictly-upper -30000 tile, rhs = identity), so the
    QK->exp->AV chain never crosses to the DVE. One ACT instruction computes
    exp for both heads through a 2-range strided AP. AV accumulates per-head
    O^T (+row sums) in PSUM over jb, lagged DELAY steps behind the exp so the
    PE never stalls on ACT. Block-causality skips all j>i blocks.
  - Normalization reads O^T straight from PSUM: reciprocal_approx_fast on the
    row-sum half + one tensor_mul -> OT in bf16 (no full-rate DVE reciprocal,
    no intermediate copy).
  - Output projection (contraction over the 4 heads = 2 pair-accumulated
    matmuls per [128n, 512d] tile) and the q/k/v projections stream through
    the attention steps as gated fillers to keep the PE dense (HAM warm).
"""

import os

import numpy as np
import ml_dtypes

B, N, D, H = 2, 2048, 1024, 16
DH = D // H  # 64
SCALE = float(D) ** -0.5
NCORES = 8
HPC = 4  # heads per core
NP = 2  # head pairs per core
IC = 512  # i-chunk width
NB = N // 128  # 16 j blocks
NCP = N // IC  # 4 i-chunks
KT = D // 128  # 8 contraction tiles
VW = 256  # v cols per (nb, pair): ones(64) | dataA(64) | ones(64) | dataB(64)
# sums land on pO partitions 0..63 for BOTH heads (reciprocal_approx_fast only
# works at base_partition 0), data on partitions 64..127
DELAY = 3
MASKV = -30000.0

_cached = {}
_last_results = None


def _build_program():
    import concourse.bacc as bacc
    import concourse.mybir as mybir
    import concourse.tile as tile

    f32 = mybir.dt.float32
    bf16 = mybir.dt.bfloat16
    EXP = mybir.ActivationFunctionType.Exp

    nc = bacc.Bacc()

    xb = nc.dram_tensor("xb", [D, N], bf16, kind="ExternalInput")  # x^T
    wq = nc.dram_tensor("wq", [D, HPC * DH], bf16, kind="ExternalInput")
    wk = nc.dram_tensor("wk", [D, HPC * DH], bf16, kind="ExternalInput")
    wv = nc.dram_tensor("wv", [D, HPC * DH], bf16, kind="ExternalInput")
    wo = nc.dram_tensor("wo", [HPC * DH, D], bf16, kind="ExternalInput")
    tri = nc.dram_tensor("tri", [128, 128], bf16, kind="ExternalInput")
    outp = nc.dram_tensor("outp", [N, D], bf16, kind="ExternalOutput")

    with tile.TileContext(nc) as tc:
        with (
            tc.tile_pool(name="const", bufs=1) as const_pool,
            tc.tile_pool(name="big", bufs=1) as big_pool,
            tc.tile_pool(name="pS", bufs=2, space="PSUM") as pS_pool,
            tc.tile_pool(name="pO", bufs=2, space="PSUM") as pO_pool,
            tc.tile_pool(name="pj", bufs=2, space="PSUM") as pj_pool,
            tc.tile_pool(name="att", bufs=4) as att_pool,
            tc.tile_pool(name="rec", bufs=4) as rec_pool,
            tc.tile_pool(name="osb", bufs=3) as osb_pool,
        ):
            # Dummy exp early: pulls the ~2.7us ACT table load off the
            # critical path (overlaps the initial DMA).
            warm = const_pool.tile([1, 8], f32, name="warm", tag="warm")
            nc.vector.memset(warm, 0.0)
            nc.scalar.activation(out=warm, in_=warm, func=EXP, scale=1.0)
            # ~3.5us of dummy matmuls while the input DMA runs: trips the HAM
            # activity window so the real stream starts at 2.4GHz, not 1.2
            wa = const_pool.tile([128, 512], bf16, name="wa", tag="wa")
            nc.vector.memset(wa, 0.0)
            for _ in range(9):
                pwarm = pj_pool.tile([128, 512], f32, name="pwarm", tag="pj")
                nc.tensor.matmul(pwarm, lhsT=wa[:, 0:128], rhs=wa, start=True, stop=True)

            # one DMA instruction per weight tensor: d-tile r lands at free-dim
            # group r of a single [128, KT*256] tile (issue cost on the sync
            # queue is ~0.6us per DMA instruction, so batching matters)
            wqa = const_pool.tile([128, KT * 256], bf16, name="wqa", tag="wqa")
            wka = const_pool.tile([128, KT * 256], bf16, name="wka", tag="wka")
            wva = const_pool.tile([128, KT * 256], bf16, name="wva", tag="wva")
            woa = const_pool.tile([128, NP * D], bf16, name="woa", tag="woa")
            nc.sync.dma_start(
                out=wva, in_=wv[:, :].rearrange("(r p) c -> p r c", r=KT)
            )
            xTall = big_pool.tile([128, KT * N], bf16, name="xTall", tag="xTall")
            xT = [xTall[:, N * r : N * (r + 1)] for r in range(KT)]
            xT4 = xTall.rearrange("p (r c) -> p r c", r=KT)
            # first column-quarter of x^T lands first so projections start early
            nc.sync.dma_start(
                out=xT4[:, :, 0:512],
                in_=xb[:, 0:512].rearrange("(r p) c -> p r c", r=KT),
            )
            nc.sync.dma_start(
                out=xT4[:, :, 512:1024],
                in_=xb[:, 512:1024].rearrange("(r p) c -> p r c", r=KT),
            )
            nc.sync.dma_start(
                out=wqa, in_=wq[:, :].rearrange("(r p) c -> p r c", r=KT)
            )
            nc.sync.dma_start(
                out=wka, in_=wk[:, :].rearrange("(r p) c -> p r c", r=KT)
            )
            tri_sb = const_pool.tile([128, 128], bf16, name="tri_sb", tag="tri_sb")
            nc.sync.dma_start(out=tri_sb, in_=tri[:, :])
            nc.sync.dma_start(
                out=xT4[:, :, 1024:2048],
                in_=xb[:, 1024:2048].rearrange("(r p) c -> p r c", r=KT),
            )
            nc.sync.dma_start(
                out=woa, in_=wo[:, :].rearrange("(p q) c -> q p c", p=NP)
            )
            wq_sb = [wqa[:, 256 * r : 256 * (r + 1)] for r in range(KT)]
            wk_sb = [wka[:, 256 * r : 256 * (r + 1)] for r in range(KT)]
            wv_sb = [wva[:, 256 * r : 256 * (r + 1)] for r in range(KT)]
            wo_sb = [woa[:, D * p : D * (p + 1)] for p in range(NP)]

            qT, kT_ = [], []
            for p in range(NP):
                qT.append(big_pool.tile([128, N], bf16, name=f"qT{p}", tag=f"qT{p}"))
                kT_.append(big_pool.tile([128, N], bf16, name=f"kT{p}", tag=f"kT{p}"))
            v_all = big_pool.tile([128, NB * NP * VW], bf16, name="v_all", tag="v_all")
            # ones for the row-sum trick; data cols overwritten by vproj copies
            nc.vector.memset(v_all, 1.0)
            OT = []
            for p in range(NP):
                OT.append(big_pool.tile([128, N], bf16, name=f"OT{p}", tag=f"OT{p}"))

            va8 = v_all.rearrange("p (n g c) -> p n g c", n=NB, g=8)

            def vproj_stream(nbs):
                for nb in nbs:
                    pv = pj_pool.tile([128, HPC * DH], f32, name="pv", tag="pj")
                    for r in range(KT):
                        nc.tensor.matmul(
                            pv,
                            lhsT=xT[r][:, 128 * nb : 128 * (nb + 1)],
                            rhs=wv_sb[r],
                            start=(r == 0),
                            stop=(r == KT - 1),
                        )
                    # head h data -> 64-col group 2h+1 (odd groups; evens stay ones)
                    pv4 = pv.rearrange("p (h c) -> p h c", h=HPC)
                    nc.vector.tensor_copy(out=va8[:, nb, 1::2, :], in_=pv4)
                    yield

            def qkproj_stream(p, cs):
                for c in cs:
                    sl = slice(IC * c, IC * (c + 1))
                    for w_sb, dst in ((wq_sb, qT[p]), (wk_sb, kT_[p])):
                        pq = pj_pool.tile([128, IC], f32, name="pq", tag="pj")
                        for r in range(KT):
                            nc.tensor.matmul(
                                pq,
                                lhsT=w_sb[r][:, 128 * p : 128 * (p + 1)],
                                rhs=xT[r][:, sl],
                                start=(r == 0),
                                stop=(r == KT - 1),
                            )
                        nc.vector.tensor_copy(out=dst[:, sl], in_=pq)
                        yield

            def outproj_stream(nbs):
                for nb in nbs:
                    nsl = slice(128 * nb, 128 * (nb + 1))
                    for s in range(2):
                        po = pj_pool.tile([128, 512], f32, name="po", tag="pj")
                        for p in range(NP):
                            nc.tensor.matmul(
                                po,
                                lhsT=OT[p][:, nsl],
                                rhs=wo_sb[p][:, 512 * s : 512 * (s + 1)],
                                start=(p == 0),
                                stop=(p == NP - 1),
                            )
                        ob = osb_pool.tile([128, 512], bf16, name="ob", tag="osb")
                        # DVE only: a scalar.copy's sem-wait would head-of-line
                        # block the exp stream on the Scalar queue
                        nc.vector.tensor_copy(out=ob, in_=po)
                        nc.gpsimd.dma_start(out=outp[nsl, 512 * s : 512 * (s + 1)], in_=ob)
                        yield

            pend = []

            def drain(n):
                while len(pend) > n:
                    pend.pop(0)()

            def attention_stream():
                for cp in range(NCP):
                    for p in range(NP):
                        pO_A = pO_pool.tile([128, IC], f32, name=f"pOA{cp}{p}", tag="pO")
                        pO_B = pO_pool.tile([128, IC], f32, name=f"pOB{cp}{p}", tag="pO")
                        jmax = 4 * cp + 4
                        for jb in range(jmax):
                            o = max(0, 128 * jb - IC * cp)
                            jsl = slice(128 * jb, 128 * (jb + 1))
                            isl = slice(IC * cp + o, IC * (cp + 1))
                            pS = pS_pool.tile([128, 2 * IC], f32, name="pS", tag="pS")
                            pexp = att_pool.tile([128, 2 * IC], bf16, name="pexp", tag="pexp")
                            # S^T pair: K=64 each, concurrent via row groups
                            nc.tensor.matmul(
                                pS[:, o:IC],
                                lhsT=kT_[p][0:64, jsl],
                                rhs=qT[p][0:64, isl],
                                start=True,
                                stop=True,
                            )
                            nc.tensor.matmul(
                                pS[:, IC + o : 2 * IC],
                                lhsT=kT_[p][64:128, jsl],
                                rhs=qT[p][64:128, isl],
                                start=True,
                                stop=True,
                            )
                            # one exp for both heads: [128, 2, IC-o] strided AP
                            src = pS.rearrange("p (h w) -> p h w", h=2)[:, :, o:]
                            dst = pexp.rearrange("p (h w) -> p h w", h=2)[:, :, o:]
                            nc.scalar.activation(out=dst, in_=src, func=EXP, scale=SCALE)
                            if 128 * jb >= IC * cp:  # diagonal block: 0/1 mask
                                for half in range(2):
                                    hb = IC * half
                                    nc.vector.tensor_mul(
                                        pexp[:, hb + o : hb + o + 128],
                                        pexp[:, hb + o : hb + o + 128],
                                        tri_sb,
                                    )

                            def av_unit(p=p, jb=jb, o=o, jmax=jmax, pO_A=pO_A, pO_B=pO_B, pexp=pexp):
                                vo = 2 * VW * jb + VW * p
                                nc.tensor.matmul(
                                    pO_A[:, o:IC],
                                    lhsT=v_all[:, vo : vo + 128],
                                    rhs=pexp[:, o:IC],
                                    start=(jb == 0),
                                    stop=(jb == jmax - 1),
                                    skip_group_check=True,
                                )
                                nc.tensor.matmul(
                                    pO_B[:, o:IC],
                                    lhsT=v_all[:, vo + 128 : vo + 256],
                                    rhs=pexp[:, IC + o : 2 * IC],
                                    start=(jb == 0),
                                    stop=(jb == jmax - 1),
                                    skip_group_check=True,
                                )

                            pend.append(av_unit)
                            drain(DELAY)
                            yield

                        # normalize straight from PSUM; OT written in bf16
                        csl = slice(IC * cp, IC * (cp + 1))
                        rec_A = rec_pool.tile([64, IC], f32, name="recA", tag="rec")
                        rec_B = rec_pool.tile([64, IC], f32, name="recB", tag="rec")

                        def recip_a(pO_A=pO_A, rec_A=rec_A):
                            nc.vector.reciprocal_approx_fast(out=rec_A, in_=pO_A[0:64, :])

                        def mul_a(pO_A=pO_A, rec_A=rec_A, p=p, csl=csl):
                            nc.vector.tensor_mul(OT[p][0:64, csl], pO_A[64:128, :], rec_A)

                        def recip_b(pO_B=pO_B, rec_B=rec_B):
                            nc.vector.reciprocal_approx_fast(out=rec_B, in_=pO_B[0:64, :])

                        def mul_b(pO_B=pO_B, rec_B=rec_B, p=p, csl=csl):
                            nc.vector.tensor_mul(OT[p][64:128, csl], pO_B[64:128, :], rec_B)

                        pend.append(recip_a)
                        pend.append(mul_a)
                        pend.append(recip_b)
                        pend.append(mul_b)

            # ---- prologue: everything attention (cp0) needs ----
            for _ in vproj_stream(range(0, 4)):
                pass
            for _ in qkproj_stream(0, [0]):
                pass
            for _ in qkproj_stream(1, [0]):
                pass

            # ---- gated fillers pulled between attention steps ----
            fillers = [
                (0, qkproj_stream(0, [1])),
                (0, qkproj_stream(1, [1])),
                (0, vproj_stream(range(4, 8))),
                (8, qkproj_stream(0, [2])),
                (8, qkproj_stream(1, [2])),
                (8, vproj_stream(range(8, 12))),
                (12, outproj_stream(range(0, 4))),
                (24, qkproj_stream(0, [3])),
                (24, qkproj_stream(1, [3])),
                (24, vproj_stream(range(12, 16))),
                (40, outproj_stream(range(4, 8))),
                (64, outproj_stream(range(8, 12))),
            ]

            def pull_filler(step):
                for i, (gate, gen) in enumerate(fillers):
                    if step < gate:
                        continue
                    if next(gen, "end") == "end":
                        fillers.pop(i)
                        continue
                    return True
                return False

            att = attention_stream()
            for step, _ in enumerate(att):
                pull_filler(step)
                if step >= 48:
                    pull_filler(step)
            drain(0)
            for _, gen in fillers:
                for _ in gen:
                    pass
            # pair 1's cp3 normalize is only issued by drain(0) above, so the
            # last output-projection blocks must stay in the epilogue. Stage
            # them in one wide SBUF tile and ship with a single DMA (per-DMA
            # issue costs ~0.6us on the queue, the dominant tail cost).
            obig = big_pool.tile([128, 4 * D], bf16, name="obig", tag="obig")
            for i, nb in enumerate(range(12, 16)):
                nsl = slice(128 * nb, 128 * (nb + 1))
                for s in range(2):
                    po = pj_pool.tile([128, 512], f32, name="po", tag="pj")
                    for p in range(NP):
                        nc.tensor.matmul(
                            po,
                            lhsT=OT[p][:, nsl],
                            rhs=wo_sb[p][:, 512 * s : 512 * (s + 1)],
                            start=(p == 0),
                            stop=(p == NP - 1),
                        )
                    dst = obig[:, D * i + 512 * s : D * i + 512 * (s + 1)]
                    if s == 0:
                        nc.vector.tensor_copy(out=dst, in_=po)
                    else:
                        nc.scalar.copy(out=dst, in_=po)
                if nb == 13:
                    nc.sync.dma_start(
                        out=outp[1536:1792, :].rearrange("(n p) c -> p n c", n=2),
                        in_=obig[:, 0 : 2 * D].rearrange("p (n c) -> p n c", n=2),
                    )
            nc.sync.dma_start(
                out=outp[1792:2048, :].rearrange("(n p) c -> p n c", n=2),
                in_=obig[:, 2 * D : 4 * D].rearrange("p (n c) -> p n c", n=2),
            )

    nc.compile()
    return nc


def kernel(x, mask, Wq, Wkv, Wout, b_out):
    global _last_results
    from concourse.bass_utils import run_bass_kernel_spmd

    bf = ml_dtypes.bfloat16
    x = np.asarray(x, dtype=np.float32)
    Wq = np.asarray(Wq, dtype=np.float32)
    Wkv = np.asarray(Wkv, dtype=np.float32)
    Wout = np.asarray(Wout, dtype=np.float32)
    b_out = np.asarray(b_out, dtype=np.float32)

    if "nc" not in _cached:
        _cached["nc"] = _build_program()
    nc = _cached["nc"]

    jj, ii = np.mgrid[0:128, 0:128]
    # pexp[j, o+c] is masked (multiplied by 0) where j > c
    tri = (jj <= ii).astype(np.float32).astype(bf)

    xTs = [np.ascontiguousarray(x[b].T).astype(bf) for b in range(B)]

    in_maps = []
    for c in range(NCORES):
        b = c // 4
        h0 = HPC * (c % 4)
        in_maps.append(
            {
                "xb": xTs[b],
                "wq": np.ascontiguousarray(Wq[:, DH * h0 : DH * (h0 + HPC)]).astype(bf),
                "wk": np.ascontiguousarray(Wkv[:, DH * h0 : DH * (h0 + HPC)]).astype(bf),
                "wv": np.ascontiguousarray(Wkv[:, D + DH * h0 : D + DH * (h0 + HPC)]).astype(bf),
                "wo": np.ascontiguousarray(Wout[DH * h0 : DH * (h0 + HPC), :]).astype(bf),
                "tri": tri,
            }
        )

    res = run_bass_kernel_spmd(
        nc,
        in_maps,
        core_ids=list(range(NCORES)),
        trace=bool(int(os.environ.get("KERNEL_TRACE", "0"))),
    )
    _last_results = res
    parts = [r["outp"] for r in res.results]
    out = np.empty((B, N, D), dtype=np.float32)
    for b in range(B):
        acc = parts[4 * b].astype(np.float32).copy()
        for c in range(4 * b + 1, 4 * b + 4):
            acc += parts[c]
        out[b] = acc + b_out[None, :]
    return out


# revision 30
# speedup vs baseline: 1.1477x; 1.0294x over previous
"""Causal multi-head attention kernel for 8 trn2 NeuronCores.

Problem: x[2,2048,1024], 16 heads of dim 64, causal softmax(q k^T / sqrt(1024)) v,
then output projection. Sharding: data-parallel over batch (4 cores per batch),
tensor-parallel over heads (4 heads per core). Each core produces a partial
output (its heads' contribution through Wout); the host sums the 4 partials per
batch and adds b_out.

Per-core device program (SPMD):
  - x arrives pre-transposed (host) as xT [d, n] in bf16; all weights bf16.
  - Projections: qT/kT [dh on partitions, n free] per head-PAIR (head A on
    partitions 0..63, head B on 64..127), v natural [n on partitions] with a
    shared ones column-block per (nb, pair): [dataA(64) | ones(64) | dataB(64)]
    so each head's AV matmul lhsT is a 128-col window whose ones half makes the
    matmul also produce softmax row-sums (A: sums on partitions 64..127,
    B: sums on partitions 0..63).
  - Attention in (cp = 512-wide i-chunk, pair, jb = 128-wide j-block) steps:
    the two heads' S^T = kT^T.qT matmuls have K=64 and run CONCURRENTLY on the
    PE via row-group tiling (head A rows 0-63, head B rows 64-127), writing the
    two halves of a pair-packed pS [128, 1024] (2 PSUM banks). The causal mask
    of the diagonal 128x128 block is applied ON THE PE as an extra accumulation
    matmul (lhsT = strictly-upper -30000 tile, rhs = identity), so the
    QK->exp->AV chain never crosses to the DVE. One ACT instruction computes
    exp for both heads through a 2-range strided AP. AV accumulates per-head
    O^T (+row sums) in PSUM over jb, lagged DELAY steps behind the exp so the
    PE never stalls on ACT. Block-causality skips all j>i blocks.
  - Normalization reads O^T straight from PSUM: reciprocal_approx_fast on the
    row-sum half + one tensor_mul -> OT in bf16 (no full-rate DVE reciprocal,
    no intermediate copy).
  - Output projection (contraction over the 4 heads = 2 pair-accumulated
    matmuls per [128n, 512d] tile) and the q/k/v projections stream through
    the attention steps as gated fillers to keep the PE dense (HAM warm).
"""

import os

import numpy as np
import ml_dtypes

B, N, D, H = 2, 2048, 1024, 16
DH = D // H  # 64
SCALE = float(D) ** -0.5
NCORES = 8
HPC = 4  # heads per core
NP = 2  # head pairs per core
IC = 512  # i-chunk width
NB = N // 128  # 16 j blocks
NCP = N // IC  # 4 i-chunks
KT = D // 128  # 8 contraction tiles
VW = 256  # v cols per (nb, pair): ones(64) | dataA(64) | ones(64) | dataB(64)
# sums land on pO partitions 0..63 for BOTH heads (reciprocal_approx_fast only
# works at base_partition 0), data on partitions 64..127
DELAY = 3
MASKV = -30000.0

_cached = {}
_last_results = None


def _build_program():
    import concourse.bacc as bacc
    import concourse.mybir as mybir
    import concourse.tile as tile

    f32 = mybir.dt.float32
    bf16 = mybir.dt.bfloat16
    EXP = mybir.ActivationFunctionType.Exp

    nc = bacc.Bacc()

    xb = nc.dram_tensor("xb", [D, N], bf16, kind="ExternalInput")  # x^T
    wq = nc.dram_tensor("wq", [D, HPC * DH], bf16, kind="ExternalInput")
    wk = nc.dram_tensor("wk", [D, HPC * DH], bf16, kind="ExternalInput")
    wv = nc.dram_tensor("wv", [D, HPC * DH], bf16, kind="ExternalInput")
    wo = nc.dram_tensor("wo", [HPC * DH, D], bf16, kind="ExternalInput")
    tri = nc.dram_tensor("tri", [128, 128], bf16, kind="ExternalInput")
    outp = nc.dram_tensor("outp", [N, D], bf16, kind="ExternalOutput")

    with tile.TileContext(nc) as tc:
        with (
            tc.tile_pool(name="const", bufs=1) as const_pool,
            tc.tile_pool(name="big", bufs=1) as big_pool,
            tc.tile_pool(name="pS", bufs=2, space="PSUM") as pS_pool,
            tc.tile_pool(name="pO", bufs=2, space="PSUM") as pO_pool,
            tc.tile_pool(name="pj", bufs=2, space="PSUM") as pj_pool,
            tc.tile_pool(name="att", bufs=4) as att_pool,
            tc.tile_pool(name="rec", bufs=4) as rec_pool,
            tc.tile_pool(name="osb", bufs=3) as osb_pool,
        ):
            # Dummy exp early: pulls the ~2.7us ACT table load off the
            # critical path (overlaps the initial DMA).
            warm = const_pool.tile([1, 8], f32, name="warm", tag="warm")
            nc.vector.memset(warm, 0.0)
            nc.scalar.activation(out=warm, in_=warm, func=EXP, scale=1.0)
            # ~3.5us of dummy matmuls while the input DMA runs: trips the HAM
            # activity window so the real stream starts at 2.4GHz, not 1.2
            wa = const_pool.tile([128, 512], bf16, name="wa", tag="wa")
            nc.vector.memset(wa, 0.0)
            for _ in range(9):
                pwarm = pj_pool.tile([128, 512], f32, name="pwarm", tag="pj")
                nc.tensor.matmul(pwarm, lhsT=wa[:, 0:128], rhs=wa, start=True, stop=True)

            # one DMA instruction per weight tensor: d-tile r lands at free-dim
            # group r of a single [128, KT*256] tile (issue cost on the sync
            # queue is ~0.6us per DMA instruction, so batching matters)
            wqa = const_pool.tile([128, KT * 256], bf16, name="wqa", tag="wqa")
            wka = const_pool.tile([128, KT * 256], bf16, name="wka", tag="wka")
            wva = const_pool.tile([128, KT * 256], bf16, name="wva", tag="wva")
            woa = const_pool.tile([128, NP * D], bf16, name="woa", tag="woa")
            nc.sync.dma_start(
                out=wva, in_=wv[:, :].rearrange("(r p) c -> p r c", r=KT)
            )
            xTall = big_pool.tile([128, KT * N], bf16, name="xTall", tag="xTall")
            xT = [xTall[:, N * r : N * (r + 1)] for r in range(KT)]
            xT4 = xTall.rearrange("p (r c) -> p r c", r=KT)
            # first column-slices of x^T land first so projections start early
            nc.sync.dma_start(
                out=xT4[:, :, 0:128],
                in_=xb[:, 0:128].rearrange("(r p) c -> p r c", r=KT),
            )
            nc.sync.dma_start(
                out=xT4[:, :, 128:512],
                in_=xb[:, 128:512].rearrange("(r p) c -> p r c", r=KT),
            )
            nc.sync.dma_start(
                out=xT4[:, :, 512:1024],
                in_=xb[:, 512:1024].rearrange("(r p) c -> p r c", r=KT),
            )
            nc.sync.dma_start(
                out=wqa, in_=wq[:, :].rearrange("(r p) c -> p r c", r=KT)
            )
            nc.sync.dma_start(
                out=wka, in_=wk[:, :].rearrange("(r p) c -> p r c", r=KT)
            )
            tri_sb = const_pool.tile([128, 128], bf16, name="tri_sb", tag="tri_sb")
            nc.sync.dma_start(out=tri_sb, in_=tri[:, :])
            nc.sync.dma_start(
                out=xT4[:, :, 1024:2048],
                in_=xb[:, 1024:2048].rearrange("(r p) c -> p r c", r=KT),
            )
            nc.sync.dma_start(
                out=woa, in_=wo[:, :].rearrange("(p q) c -> q p c", p=NP)
            )
            wq_sb = [wqa[:, 256 * r : 256 * (r + 1)] for r in range(KT)]
            wk_sb = [wka[:, 256 * r : 256 * (r + 1)] for r in range(KT)]
            wv_sb = [wva[:, 256 * r : 256 * (r + 1)] for r in range(KT)]
            wo_sb = [woa[:, D * p : D * (p + 1)] for p in range(NP)]

            qT, kT_ = [], []
            for p in range(NP):
                qT.append(big_pool.tile([128, N], bf16, name=f"qT{p}", tag=f"qT{p}"))
                kT_.append(big_pool.tile([128, N], bf16, name=f"kT{p}", tag=f"kT{p}"))
            v_all = big_pool.tile([128, NB * NP * VW], bf16, name="v_all", tag="v_all")
            # ones for the row-sum trick; data cols overwritten by vproj copies
            nc.vector.memset(v_all, 1.0)
            OT = []
            for p in range(NP):
                OT.append(big_pool.tile([128, N], bf16, name=f"OT{p}", tag=f"OT{p}"))

            va8 = v_all.rearrange("p (n g c) -> p n g c", n=NB, g=8)

            def vproj_stream(nbs):
                for nb in nbs:
                    pv = pj_pool.tile([128, HPC * DH], f32, name="pv", tag="pj")
                    for r in range(KT):
                        nc.tensor.matmul(
                            pv,
                            lhsT=xT[r][:, 128 * nb : 128 * (nb + 1)],
                            rhs=wv_sb[r],
                            start=(r == 0),
                            stop=(r == KT - 1),
                        )
                    # head h data -> 64-col group 2h+1 (odd groups; evens stay ones)
                    pv4 = pv.rearrange("p (h c) -> p h c", h=HPC)
                    nc.vector.tensor_copy(out=va8[:, nb, 1::2, :], in_=pv4)
                    yield

            def qkproj_stream(p, cs):
                for c in cs:
                    sl = slice(IC * c, IC * (c + 1))
                    for w_sb, dst in ((wq_sb, qT[p]), (wk_sb, kT_[p])):
                        pq = pj_pool.tile([128, IC], f32, name="pq", tag="pj")
                        for r in range(KT):
                            nc.tensor.matmul(
                                pq,
                                lhsT=w_sb[r][:, 128 * p : 128 * (p + 1)],
                                rhs=xT[r][:, sl],
                                start=(r == 0),
                                stop=(r == KT - 1),
                            )
                        nc.vector.tensor_copy(out=dst[:, sl], in_=pq)
                        yield

            def outproj_stream(nbs):
                for nb in nbs:
                    nsl = slice(128 * nb, 128 * (nb + 1))
                    for s in range(2):
                        po = pj_pool.tile([128, 512], f32, name="po", tag="pj")
                        for p in range(NP):
                            nc.tensor.matmul(
                                po,
                                lhsT=OT[p][:, nsl],
                                rhs=wo_sb[p][:, 512 * s : 512 * (s + 1)],
                                start=(p == 0),
                                stop=(p == NP - 1),
                            )
                        ob = osb_pool.tile([128, 512], bf16, name="ob", tag="osb")
                        # DVE only: a scalar.copy's sem-wait would head-of-line
                        # block the exp stream on the Scalar queue
                        nc.vector.tensor_copy(out=ob, in_=po)
                        nc.gpsimd.dma_start(out=outp[nsl, 512 * s : 512 * (s + 1)], in_=ob)
                        yield

            pend = []

            def drain(n):
                while len(pend) > n:
                    pend.pop(0)()

            def attention_stream():
                for cp in range(NCP):
                    for p in range(NP):
                        pO_A = pO_pool.tile([128, IC], f32, name=f"pOA{cp}{p}", tag="pO")
                        pO_B = pO_pool.tile([128, IC], f32, name=f"pOB{cp}{p}", tag="pO")
                        jmax = 4 * cp + 4
                        for jb in range(jmax):
                            o = max(0, 128 * jb - IC * cp)
                            jsl = slice(128 * jb, 128 * (jb + 1))
                            isl = slice(IC * cp + o, IC * (cp + 1))
                            pS = pS_pool.tile([128, 2 * IC], f32, name="pS", tag="pS")
                            pexp = att_pool.tile([128, 2 * IC], bf16, name="pexp", tag="pexp")
                            # S^T pair: K=64 each, concurrent via row groups
                            nc.tensor.matmul(
                                pS[:, o:IC],
                                lhsT=kT_[p][0:64, jsl],
                                rhs=qT[p][0:64, isl],
                                start=True,
                                stop=True,
                            )
                            nc.tensor.matmul(
                                pS[:, IC + o : 2 * IC],
                                lhsT=kT_[p][64:128, jsl],
                                rhs=qT[p][64:128, isl],
                                start=True,
                                stop=True,
                            )
                            # one exp for both heads: [128, 2, IC-o] strided AP
                            src = pS.rearrange("p (h w) -> p h w", h=2)[:, :, o:]
                            dst = pexp.rearrange("p (h w) -> p h w", h=2)[:, :, o:]
                            nc.scalar.activation(out=dst, in_=src, func=EXP, scale=SCALE)
                            if 128 * jb >= IC * cp:  # diagonal block: 0/1 mask
                                for half in range(2):
                                    hb = IC * half
                                    nc.vector.tensor_mul(
                                        pexp[:, hb + o : hb + o + 128],
                                        pexp[:, hb + o : hb + o + 128],
                                        tri_sb,
                                    )

                            def av_unit(p=p, jb=jb, o=o, jmax=jmax, pO_A=pO_A, pO_B=pO_B, pexp=pexp):
                                vo = 2 * VW * jb + VW * p
                                nc.tensor.matmul(
                                    pO_A[:, o:IC],
                                    lhsT=v_all[:, vo : vo + 128],
                                    rhs=pexp[:, o:IC],
                                    start=(jb == 0),
                                    stop=(jb == jmax - 1),
                                    skip_group_check=True,
                                )
                                nc.tensor.matmul(
                                    pO_B[:, o:IC],
                                    lhsT=v_all[:, vo + 128 : vo + 256],
                                    rhs=pexp[:, IC + o : 2 * IC],
                                    start=(jb == 0),
                                    stop=(jb == jmax - 1),
                                    skip_group_check=True,
                                )

                            pend.append(av_unit)
                            drain(DELAY)
                            yield

                        # normalize straight from PSUM; OT written in bf16
                        csl = slice(IC * cp, IC * (cp + 1))
                        rec_A = rec_pool.tile([64, IC], f32, name="recA", tag="rec")
                        rec_B = rec_pool.tile([64, IC], f32, name="recB", tag="rec")

                        def recip_a(pO_A=pO_A, rec_A=rec_A):
                            nc.vector.reciprocal_approx_fast(out=rec_A, in_=pO_A[0:64, :])

                        def mul_a(pO_A=pO_A, rec_A=rec_A, p=p, csl=csl):
                            nc.vector.tensor_mul(OT[p][0:64, csl], pO_A[64:128, :], rec_A)

                        def recip_b(pO_B=pO_B, rec_B=rec_B):
                            nc.vector.reciprocal_approx_fast(out=rec_B, in_=pO_B[0:64, :])

                        def mul_b(pO_B=pO_B, rec_B=rec_B, p=p, csl=csl):
                            nc.vector.tensor_mul(OT[p][64:128, csl], pO_B[64:128, :], rec_B)

                        pend.append(recip_a)
                        pend.append(mul_a)
                        pend.append(recip_b)
                        pend.append(mul_b)

            # ---- prologue: everything attention (cp0) needs ----
            for _ in vproj_stream(range(0, 4)):
                pass
            for _ in qkproj_stream(0, [0]):
                pass
            for _ in qkproj_stream(1, [0]):
                pass

            # ---- gated fillers pulled between attention steps ----
            fillers = [
                (0, qkproj_stream(0, [1])),
                (0, qkproj_stream(1, [1])),
                (0, vproj_stream(range(4, 8))),
                (8, qkproj_stream(0, [2])),
                (8, qkproj_stream(1, [2])),
                (8, vproj_stream(range(8, 12))),
                (12, outproj_stream(range(0, 4))),
                (24, qkproj_stream(0, [3])),
                (24, qkproj_stream(1, [3])),
                (24, vproj_stream(range(12, 16))),
                (48, outproj_stream(range(4, 8))),
                (64, outproj_stream(range(8, 12))),
            ]

            def pull_filler(step):
                for i, (gate, gen) in enumerate(fillers):
                    if step < gate:
                        continue
                    if next(gen, "end") == "end":
                        fillers.pop(i)
                        continue
                    return True
                return False

            att = attention_stream()
            for step, _ in enumerate(att):
                pull_filler(step)
                if step >= 48:
                    pull_filler(step)
            drain(0)
            for _, gen in fillers:
                for _ in gen:
                    pass
            # pair 1's cp3 normalize is only issued by drain(0) above, so the
            # last output-projection blocks must stay in the epilogue. Stage
            # them in one wide SBUF tile and ship with a single DMA (per-DMA
            # issue costs ~0.6us on the queue, the dominant tail cost).
            obig = big_pool.tile([128, 4 * D], bf16, name="obig", tag="obig")
            for i, nb in enumerate(range(12, 16)):
                nsl = slice(128 * nb, 128 * (nb + 1))
                for s in range(2):
                    po = pj_pool.tile([128, 512], f32, name="po", tag="pj")
                    for p in range(NP):
                        nc.tensor.matmul(
                            po,
                            lhsT=OT[p][:, nsl],
                            rhs=wo_sb[p][:, 512 * s : 512 * (s + 1)],
                            start=(p == 0),
                            stop=(p == NP - 1),
                        )
                    dst = obig[:, D * i + 512 * s : D * i + 512 * (s + 1)]
                    if s == 0:
                        nc.vector.tensor_copy(out=dst, in_=po)
                    else:
                        nc.scalar.copy(out=dst, in_=po)
                if nb == 13:
                    nc.sync.dma_start(
                        out=outp[1536:1792, :].rearrange("(n p) c -> p n c", n=2),
                        in_=obig[:, 0 : 2 * D].rearrange("p (n c) -> p n c", n=2),
                    )
            nc.sync.dma_start(
                out=outp[1792:2048, :].rearrange("(n p) c -> p n c", n=2),
                in_=obig[:, 2 * D : 4 * D].rearrange("p (n c) -> p n c", n=2),
            )

    nc.compile()
    return nc


def kernel(x, mask, Wq, Wkv, Wout, b_out):
    global _last_results
    from concourse.bass_utils import run_bass_kernel_spmd

    bf = ml_dtypes.bfloat16
    x = np.asarray(x, dtype=np.float32)
    Wq = np.asarray(Wq, dtype=np.float32)
    Wkv = np.asarray(Wkv, dtype=np.float32)
    Wout = np.asarray(Wout, dtype=np.float32)
    b_out = np.asarray(b_out, dtype=np.float32)

    if "nc" not in _cached:
        _cached["nc"] = _build_program()
    nc = _cached["nc"]

    jj, ii = np.mgrid[0:128, 0:128]
    # pexp[j, o+c] is masked (multiplied by 0) where j > c
    tri = (jj <= ii).astype(np.float32).astype(bf)

    xTs = [np.ascontiguousarray(x[b].T).astype(bf) for b in range(B)]

    in_maps = []
    for c in range(NCORES):
        b = c // 4
        h0 = HPC * (c % 4)
        in_maps.append(
            {
                "xb": xTs[b],
                "wq": np.ascontiguousarray(Wq[:, DH * h0 : DH * (h0 + HPC)]).astype(bf),
                "wk": np.ascontiguousarray(Wkv[:, DH * h0 : DH * (h0 + HPC)]).astype(bf),
                "wv": np.ascontiguousarray(Wkv[:, D + DH * h0 : D + DH * (h0 + HPC)]).astype(bf),
                "wo": np.ascontiguousarray(Wout[DH * h0 : DH * (h0 + HPC), :]).astype(bf),
                "tri": tri,
            }
        )

    res = run_bass_kernel_spmd(
        nc,
        in_maps,
        core_ids=list(range(NCORES)),
        trace=bool(int(os.environ.get("KERNEL_TRACE", "0"))),
    )
    _last_results = res
    parts = [r["outp"] for r in res.results]
    out = np.empty((B, N, D), dtype=np.float32)
    for b in range(B):
        acc = parts[4 * b].astype(np.float32).copy()
        for c in range(4 * b + 1, 4 * b + 4):
            acc += parts[c]
        out[b] = acc + b_out[None, :]
    return out


# revision 32
# speedup vs baseline: 1.1551x; 1.0065x over previous
"""Causal multi-head attention kernel for 8 trn2 NeuronCores.

Problem: x[2,2048,1024], 16 heads of dim 64, causal softmax(q k^T / sqrt(1024)) v,
then output projection. Sharding: data-parallel over batch (4 cores per batch),
tensor-parallel over heads (4 heads per core). Each core produces a partial
output (its heads' contribution through Wout); the host sums the 4 partials per
batch and adds b_out.

Per-core device program (SPMD):
  - x arrives pre-transposed (host) as xT [d, n] in bf16; all weights bf16.
  - Projections: qT/kT [dh on partitions, n free] per head-PAIR (head A on
    partitions 0..63, head B on 64..127), v natural [n on partitions] with a
    shared ones column-block per (nb, pair): [dataA(64) | ones(64) | dataB(64)]
    so each head's AV matmul lhsT is a 128-col window whose ones half makes the
    matmul also produce softmax row-sums (A: sums on partitions 64..127,
    B: sums on partitions 0..63).
  - Attention in (cp = 512-wide i-chunk, pair, jb = 128-wide j-block) steps:
    the two heads' S^T = kT^T.qT matmuls have K=64 and run CONCURRENTLY on the
    PE via row-group tiling (head A rows 0-63, head B rows 64-127), writing the
    two halves of a pair-packed pS [128, 1024] (2 PSUM banks). The causal mask
    of the diagonal 128x128 block is applied ON THE PE as an extra accumulation
    matmul (lhsT = strictly-upper -30000 tile, rhs = identity), so the
    QK->exp->AV chain never crosses to the DVE. One ACT instruction computes
    exp for both heads through a 2-range strided AP. AV accumulates per-head
    O^T (+row sums) in PSUM over jb, lagged DELAY steps behind the exp so the
    PE never stalls on ACT. Block-causality skips all j>i blocks.
  - Normalization reads O^T straight from PSUM: reciprocal_approx_fast on the
    row-sum half + one tensor_mul -> OT in bf16 (no full-rate DVE reciprocal,
    no intermediate copy).
  - Output projection (contraction over the 4 heads = 2 pair-accumulated
    matmuls per [128n, 512d] tile) and the q/k/v projections stream through
    the attention steps as gated fillers to keep the PE dense (HAM warm).
"""

import os

import numpy as np
import ml_dtypes

B, N, D, H = 2, 2048, 1024, 16
DH = D // H  # 64
SCALE = float(D) ** -0.5
NCORES = 8
HPC = 4  # heads per core
NP = 2  # head pairs per core
IC = 512  # i-chunk width
NB = N // 128  # 16 j blocks
NCP = N // IC  # 4 i-chunks
KT = D // 128  # 8 contraction tiles
VW = 256  # v cols per (nb, pair): ones(64) | dataA(64) | ones(64) | dataB(64)
# sums land on pO partitions 0..63 for BOTH heads (reciprocal_approx_fast only
# works at base_partition 0), data on partitions 64..127
DELAY = 3
MASKV = -30000.0

_cached = {}
_last_results = None


def _build_program():
    import concourse.bacc as bacc
    import concourse.mybir as mybir
    import concourse.tile as tile

    f32 = mybir.dt.float32
    bf16 = mybir.dt.bfloat16
    EXP = mybir.ActivationFunctionType.Exp

    nc = bacc.Bacc()

    xb = nc.dram_tensor("xb", [D, N], bf16, kind="ExternalInput")  # x^T
    wq = nc.dram_tensor("wq", [D, HPC * DH], bf16, kind="ExternalInput")
    wk = nc.dram_tensor("wk", [D, HPC * DH], bf16, kind="ExternalInput")
    wv = nc.dram_tensor("wv", [D, HPC * DH], bf16, kind="ExternalInput")
    wo = nc.dram_tensor("wo", [HPC * DH, D], bf16, kind="ExternalInput")
    tri = nc.dram_tensor("tri", [128, 128], bf16, kind="ExternalInput")
    outp = nc.dram_tensor("outp", [N, D], bf16, kind="ExternalOutput")

    with tile.TileContext(nc) as tc:
        with (
            tc.tile_pool(name="const", bufs=1) as const_pool,
            tc.tile_pool(name="big", bufs=1) as big_pool,
            tc.tile_pool(name="pS", bufs=2, space="PSUM") as pS_pool,
            tc.tile_pool(name="pO", bufs=2, space="PSUM") as pO_pool,
            tc.tile_pool(name="pj", bufs=2, space="PSUM") as pj_pool,
            tc.tile_pool(name="att", bufs=5) as att_pool,
            tc.tile_pool(name="rec", bufs=4) as rec_pool,
            tc.tile_pool(name="osb", bufs=3) as osb_pool,
        ):
            # Dummy exp early: pulls the ~2.7us ACT table load off the
            # critical path (overlaps the initial DMA).
            warm = const_pool.tile([1, 8], f32, name="warm", tag="warm")
            nc.vector.memset(warm, 0.0)
            nc.scalar.activation(out=warm, in_=warm, func=EXP, scale=1.0)
            # ~3.5us of dummy matmuls while the input DMA runs: trips the HAM
            # activity window so the real stream starts at 2.4GHz, not 1.2
            wa = const_pool.tile([128, 512], bf16, name="wa", tag="wa")
            nc.vector.memset(wa, 0.0)
            for _ in range(9):
                pwarm = pj_pool.tile([128, 512], f32, name="pwarm", tag="pj")
                nc.tensor.matmul(pwarm, lhsT=wa[:, 0:128], rhs=wa, start=True, stop=True)

            # one DMA instruction per weight tensor: d-tile r lands at free-dim
            # group r of a single [128, KT*256] tile (issue cost on the sync
            # queue is ~0.6us per DMA instruction, so batching matters)
            wqa = const_pool.tile([128, KT * 256], bf16, name="wqa", tag="wqa")
            wka = const_pool.tile([128, KT * 256], bf16, name="wka", tag="wka")
            wva = const_pool.tile([128, KT * 256], bf16, name="wva", tag="wva")
            woa = const_pool.tile([128, NP * D], bf16, name="woa", tag="woa")
            nc.sync.dma_start(
                out=wva, in_=wv[:, :].rearrange("(r p) c -> p r c", r=KT)
            )
            xTall = big_pool.tile([128, KT * N], bf16, name="xTall", tag="xTall")
            xT = [xTall[:, N * r : N * (r + 1)] for r in range(KT)]
            xT4 = xTall.rearrange("p (r c) -> p r c", r=KT)
            # first column-slices of x^T land first so projections start early
            nc.sync.dma_start(
                out=xT4[:, :, 0:128],
                in_=xb[:, 0:128].rearrange("(r p) c -> p r c", r=KT),
            )
            nc.sync.dma_start(
                out=xT4[:, :, 128:512],
                in_=xb[:, 128:512].rearrange("(r p) c -> p r c", r=KT),
            )
            nc.sync.dma_start(
                out=xT4[:, :, 512:1024],
                in_=xb[:, 512:1024].rearrange("(r p) c -> p r c", r=KT),
            )
            nc.sync.dma_start(
                out=wqa, in_=wq[:, :].rearrange("(r p) c -> p r c", r=KT)
            )
            nc.sync.dma_start(
                out=wka, in_=wk[:, :].rearrange("(r p) c -> p r c", r=KT)
            )
            tri_sb = const_pool.tile([128, 128], bf16, name="tri_sb", tag="tri_sb")
            nc.sync.dma_start(out=tri_sb, in_=tri[:, :])
            nc.sync.dma_start(
                out=xT4[:, :, 1024:2048],
                in_=xb[:, 1024:2048].rearrange("(r p) c -> p r c", r=KT),
            )
            nc.sync.dma_start(
                out=woa, in_=wo[:, :].rearrange("(p q) c -> q p c", p=NP)
            )
            wq_sb = [wqa[:, 256 * r : 256 * (r + 1)] for r in range(KT)]
            wk_sb = [wka[:, 256 * r : 256 * (r + 1)] for r in range(KT)]
            wv_sb = [wva[:, 256 * r : 256 * (r + 1)] for r in range(KT)]
            wo_sb = [woa[:, D * p : D * (p + 1)] for p in range(NP)]

            qT, kT_ = [], []
            for p in range(NP):
                qT.append(big_pool.tile([128, N], bf16, name=f"qT{p}", tag=f"qT{p}"))
                kT_.append(big_pool.tile([128, N], bf16, name=f"kT{p}", tag=f"kT{p}"))
            v_all = big_pool.tile([128, NB * NP * VW], bf16, name="v_all", tag="v_all")
            # ones for the row-sum trick; data cols overwritten by vproj copies
            nc.vector.memset(v_all, 1.0)
            OT = []
            for p in range(NP):
                OT.append(big_pool.tile([128, N], bf16, name=f"OT{p}", tag=f"OT{p}"))

            va8 = v_all.rearrange("p (n g c) -> p n g c", n=NB, g=8)

            def vproj_stream(nbs):
                for nb in nbs:
                    pv = pj_pool.tile([128, HPC * DH], f32, name="pv", tag="pj")
                    for r in range(KT):
                        nc.tensor.matmul(
                            pv,
                            lhsT=xT[r][:, 128 * nb : 128 * (nb + 1)],
                            rhs=wv_sb[r],
                            start=(r == 0),
                            stop=(r == KT - 1),
                        )
                    # head h data -> 64-col group 2h+1 (odd groups; evens stay ones)
                    pv4 = pv.rearrange("p (h c) -> p h c", h=HPC)
                    nc.vector.tensor_copy(out=va8[:, nb, 1::2, :], in_=pv4)
                    yield

            def qkproj_stream(p, cs):
                for c in cs:
                    sl = slice(IC * c, IC * (c + 1))
                    for w_sb, dst in ((wq_sb, qT[p]), (wk_sb, kT_[p])):
                        pq = pj_pool.tile([128, IC], f32, name="pq", tag="pj")
                        for r in range(KT):
                            nc.tensor.matmul(
                                pq,
                                lhsT=w_sb[r][:, 128 * p : 128 * (p + 1)],
                                rhs=xT[r][:, sl],
                                start=(r == 0),
                                stop=(r == KT - 1),
                            )
                        nc.vector.tensor_copy(out=dst[:, sl], in_=pq)
                        yield

            def outproj_stream(nbs):
                for nb in nbs:
                    nsl = slice(128 * nb, 128 * (nb + 1))
                    for s in range(2):
                        po = pj_pool.tile([128, 512], f32, name="po", tag="pj")
                        for p in range(NP):
                            nc.tensor.matmul(
                                po,
                                lhsT=OT[p][:, nsl],
                                rhs=wo_sb[p][:, 512 * s : 512 * (s + 1)],
                                start=(p == 0),
                                stop=(p == NP - 1),
                            )
                        ob = osb_pool.tile([128, 512], bf16, name="ob", tag="osb")
                        # DVE only: a scalar.copy's sem-wait would head-of-line
                        # block the exp stream on the Scalar queue
                        nc.vector.tensor_copy(out=ob, in_=po)
                        nc.gpsimd.dma_start(out=outp[nsl, 512 * s : 512 * (s + 1)], in_=ob)
                        yield

            pend = []

            def drain(n):
                while len(pend) > n:
                    pend.pop(0)()

            def attention_stream():
                for cp in range(NCP):
                    for p in range(NP):
                        pO_A = pO_pool.tile([128, IC], f32, name=f"pOA{cp}{p}", tag="pO")
                        pO_B = pO_pool.tile([128, IC], f32, name=f"pOB{cp}{p}", tag="pO")
                        jmax = 4 * cp + 4
                        for jb in range(jmax):
                            o = max(0, 128 * jb - IC * cp)
                            jsl = slice(128 * jb, 128 * (jb + 1))
                            isl = slice(IC * cp + o, IC * (cp + 1))
                            pS = pS_pool.tile([128, 2 * IC], f32, name="pS", tag="pS")
                            pexp = att_pool.tile([128, 2 * IC], bf16, name="pexp", tag="pexp")
                            # S^T pair: K=64 each, concurrent via row groups
                            nc.tensor.matmul(
                                pS[:, o:IC],
                                lhsT=kT_[p][0:64, jsl],
                                rhs=qT[p][0:64, isl],
                                start=True,
                                stop=True,
                            )
                            nc.tensor.matmul(
                                pS[:, IC + o : 2 * IC],
                                lhsT=kT_[p][64:128, jsl],
                                rhs=qT[p][64:128, isl],
                                start=True,
                                stop=True,
                            )
                            # one exp for both heads: [128, 2, IC-o] strided AP
                            src = pS.rearrange("p (h w) -> p h w", h=2)[:, :, o:]
                            dst = pexp.rearrange("p (h w) -> p h w", h=2)[:, :, o:]
                            nc.scalar.activation(out=dst, in_=src, func=EXP, scale=SCALE)
                            if 128 * jb >= IC * cp:  # diagonal block: 0/1 mask
                                for half in range(2):
                                    hb = IC * half
                                    nc.vector.tensor_mul(
                                        pexp[:, hb + o : hb + o + 128],
                                        pexp[:, hb + o : hb + o + 128],
                                        tri_sb,
                                    )

                            def av_unit(p=p, jb=jb, o=o, jmax=jmax, pO_A=pO_A, pO_B=pO_B, pexp=pexp):
                                vo = 2 * VW * jb + VW * p
                                nc.tensor.matmul(
                                    pO_A[:, o:IC],
                                    lhsT=v_all[:, vo : vo + 128],
                                    rhs=pexp[:, o:IC],
                                    start=(jb == 0),
                                    stop=(jb == jmax - 1),
                                    skip_group_check=True,
                                )
                                nc.tensor.matmul(
                                    pO_B[:, o:IC],
                                    lhsT=v_all[:, vo + 128 : vo + 256],
                                    rhs=pexp[:, IC + o : 2 * IC],
                                    start=(jb == 0),
                                    stop=(jb == jmax - 1),
                                    skip_group_check=True,
                                )

                            pend.append(av_unit)
                            drain(DELAY)
                            yield

                        # normalize straight from PSUM; OT written in bf16
                        csl = slice(IC * cp, IC * (cp + 1))
                        rec_A = rec_pool.tile([64, IC], f32, name="recA", tag="rec")
                        rec_B = rec_pool.tile([64, IC], f32, name="recB", tag="rec")

                        def recip_a(pO_A=pO_A, rec_A=rec_A):
                            nc.vector.reciprocal_approx_fast(out=rec_A, in_=pO_A[0:64, :])

                        def mul_a(pO_A=pO_A, rec_A=rec_A, p=p, csl=csl):
                            nc.vector.tensor_mul(OT[p][0:64, csl], pO_A[64:128, :], rec_A)

                        def recip_b(pO_B=pO_B, rec_B=rec_B):
                            nc.vector.reciprocal_approx_fast(out=rec_B, in_=pO_B[0:64, :])

                        def mul_b(pO_B=pO_B, rec_B=rec_B, p=p, csl=csl):
                            nc.vector.tensor_mul(OT[p][64:128, csl], pO_B[64:128, :], rec_B)

                        pend.append(recip_a)
                        pend.append(mul_a)
                        pend.append(recip_b)
                        pend.append(mul_b)

            # ---- prologue: everything attention (cp0) needs ----
            for _ in vproj_stream(range(0, 4)):
                pass
            for _ in qkproj_stream(0, [0]):
                pass
            for _ in qkproj_stream(1, [0]):
                pass

            # ---- gated fillers pulled between attention steps ----
            fillers = [
                (0, qkproj_stream(0, [1])),
                (0, qkproj_stream(1, [1])),
                (0, vproj_stream(range(4, 8))),
                (8, qkproj_stream(0, [2])),
                (8, qkproj_stream(1, [2])),
                (8, vproj_stream(range(8, 12))),
                (12, outproj_stream(range(0, 4))),
                (24, qkproj_stream(0, [3])),
                (24, qkproj_stream(1, [3])),
                (24, vproj_stream(range(12, 16))),
                (48, outproj_stream(range(4, 8))),
                (64, outproj_stream(range(8, 12))),
            ]

            def pull_filler(step):
                for i, (gate, gen) in enumerate(fillers):
                    if step < gate:
                        continue
                    if next(gen, "end") == "end":
                        fillers.pop(i)
                        continue
                    return True
                return False

            att = attention_stream()
            for step, _ in enumerate(att):
                pull_filler(step)
                if step >= 48:
                    pull_filler(step)
            drain(0)
            for _, gen in fillers:
                for _ in gen:
                    pass
            # pair 1's cp3 normalize is only issued by drain(0) above, so the
            # last output-projection blocks must stay in the epilogue. Stage
            # them in one wide SBUF tile and ship with a single DMA (per-DMA
            # issue costs ~0.6us on the queue, the dominant tail cost).
            obig = big_pool.tile([128, 4 * D], bf16, name="obig", tag="obig")
            for i, nb in enumerate(range(12, 16)):
                nsl = slice(128 * nb, 128 * (nb + 1))
                for s in range(2):
                    po = pj_pool.tile([128, 512], f32, name="po", tag="pj")
                    for p in range(NP):
                        nc.tensor.matmul(
                            po,
                            lhsT=OT[p][:, nsl],
                            rhs=wo_sb[p][:, 512 * s : 512 * (s + 1)],
                            start=(p == 0),
                            stop=(p == NP - 1),
                        )
                    dst = obig[:, D * i + 512 * s : D * i + 512 * (s + 1)]
                    if s == 0:
                        nc.vector.tensor_copy(out=dst, in_=po)
                    else:
                        nc.scalar.copy(out=dst, in_=po)
                nc.sync.dma_start(
                    out=outp[128 * nb : 128 * (nb + 1), :],
                    in_=obig[:, D * i : D * (i + 1)],
                )

    nc.compile()
    return nc


def kernel(x, mask, Wq, Wkv, Wout, b_out):
    global _last_results
    from concourse.bass_utils import run_bass_kernel_spmd

    bf = ml_dtypes.bfloat16
    x = np.asarray(x, dtype=np.float32)
    Wq = np.asarray(Wq, dtype=np.float32)
    Wkv = np.asarray(Wkv, dtype=np.float32)
    Wout = np.asarray(Wout, dtype=np.float32)
    b_out = np.asarray(b_out, dtype=np.float32)

    if "nc" not in _cached:
        _cached["nc"] = _build_program()
    nc = _cached["nc"]

    jj, ii = np.mgrid[0:128, 0:128]
    # pexp[j, o+c] is masked (multiplied by 0) where j > c
    tri = (jj <= ii).astype(np.float32).astype(bf)

    xTs = [np.ascontiguousarray(x[b].T).astype(bf) for b in range(B)]

    in_maps = []
    for c in range(NCORES):
        b = c // 4
        h0 = HPC * (c % 4)
        in_maps.append(
            {
                "xb": xTs[b],
                "wq": np.ascontiguousarray(Wq[:, DH * h0 : DH * (h0 + HPC)]).astype(bf),
                "wk": np.ascontiguousarray(Wkv[:, DH * h0 : DH * (h0 + HPC)]).astype(bf),
                "wv": np.ascontiguousarray(Wkv[:, D + DH * h0 : D + DH * (h0 + HPC)]).astype(bf),
                "wo": np.ascontiguousarray(Wout[DH * h0 : DH * (h0 + HPC), :]).astype(bf),
                "tri": tri,
            }
        )

    res = run_bass_kernel_spmd(
        nc,
        in_maps,
        core_ids=list(range(NCORES)),
        trace=bool(int(os.environ.get("KERNEL_TRACE", "0"))),
    )
    _last_results = res
    parts = [r["outp"] for r in res.results]
    out = np.empty((B, N, D), dtype=np.float32)
    for b in range(B):
        acc = parts[4 * b].astype(np.float32).copy()
        for c in range(4 * b + 1, 4 * b + 4):
            acc += parts[c]
        out[b] = acc + b_out[None, :]
    return out


# revision 34
# speedup vs baseline: 1.1662x; 1.0096x over previous
"""Causal multi-head attention kernel for 8 trn2 NeuronCores.

Problem: x[2,2048,1024], 16 heads of dim 64, causal softmax(q k^T / sqrt(1024)) v,
then output projection. Sharding: data-parallel over batch (4 cores per batch),
tensor-parallel over heads (4 heads per core). Each core produces a partial
output (its heads' contribution through Wout); the host sums the 4 partials per
batch and adds b_out.

Per-core device program (SPMD):
  - x arrives pre-transposed (host) as xT [d, n] in bf16; all weights bf16.
  - Projections: qT/kT [dh on partitions, n free] per head-PAIR (head A on
    partitions 0..63, head B on 64..127), v natural [n on partitions] with a
    shared ones column-block per (nb, pair): [dataA(64) | ones(64) | dataB(64)]
    so each head's AV matmul lhsT is a 128-col window whose ones half makes the
    matmul also produce softmax row-sums (A: sums on partitions 64..127,
    B: sums on partitions 0..63).
  - Attention in (cp = 512-wide i-chunk, pair, jb = 128-wide j-block) steps:
    the two heads' S^T = kT^T.qT matmuls have K=64 and run CONCURRENTLY on the
    PE via row-group tiling (head A rows 0-63, head B rows 64-127), writing the
    two halves of a pair-packed pS [128, 1024] (2 PSUM banks). The causal mask
    of the diagonal 128x128 block is applied ON THE PE as an extra accumulation
    matmul (lhsT = strictly-upper -30000 tile, rhs = identity), so the
    QK->exp->AV chain never crosses to the DVE. One ACT instruction computes
    exp for both heads through a 2-range strided AP. AV accumulates per-head
    O^T (+row sums) in PSUM over jb, lagged DELAY steps behind the exp so the
    PE never stalls on ACT. Block-causality skips all j>i blocks.
  - Normalization reads O^T straight from PSUM: reciprocal_approx_fast on the
    row-sum half + one tensor_mul -> OT in bf16 (no full-rate DVE reciprocal,
    no intermediate copy).
  - Output projection (contraction over the 4 heads = 2 pair-accumulated
    matmuls per [128n, 512d] tile) and the q/k/v projections stream through
    the attention steps as gated fillers to keep the PE dense (HAM warm).
"""

import os

import numpy as np
import ml_dtypes

B, N, D, H = 2, 2048, 1024, 16
DH = D // H  # 64
SCALE = float(D) ** -0.5
NCORES = 8
HPC = 4  # heads per core
NP = 2  # head pairs per core
IC = 512  # i-chunk width
NB = N // 128  # 16 j blocks
NCP = N // IC  # 4 i-chunks
KT = D // 128  # 8 contraction tiles
VW = 256  # v cols per (nb, pair): ones(64) | dataA(64) | ones(64) | dataB(64)
# sums land on pO partitions 0..63 for BOTH heads (reciprocal_approx_fast only
# works at base_partition 0), data on partitions 64..127
DELAY = 3
MASKV = -30000.0

_cached = {}
_last_results = None


def _build_program():
    import concourse.bacc as bacc
    import concourse.mybir as mybir
    import concourse.tile as tile

    f32 = mybir.dt.float32
    bf16 = mybir.dt.bfloat16
    EXP = mybir.ActivationFunctionType.Exp

    nc = bacc.Bacc()

    xb = nc.dram_tensor("xb", [D, N], bf16, kind="ExternalInput")  # x^T
    wq = nc.dram_tensor("wq", [D, HPC * DH], bf16, kind="ExternalInput")
    wk = nc.dram_tensor("wk", [D, HPC * DH], bf16, kind="ExternalInput")
    wv = nc.dram_tensor("wv", [D, HPC * DH], bf16, kind="ExternalInput")
    wo = nc.dram_tensor("wo", [HPC * DH, D], bf16, kind="ExternalInput")
    tri = nc.dram_tensor("tri", [128, 128], bf16, kind="ExternalInput")
    outp = nc.dram_tensor("outp", [N, D], bf16, kind="ExternalOutput")

    with tile.TileContext(nc) as tc:
        with (
            tc.tile_pool(name="const", bufs=1) as const_pool,
            tc.tile_pool(name="big", bufs=1) as big_pool,
            tc.tile_pool(name="pS", bufs=2, space="PSUM") as pS_pool,
            tc.tile_pool(name="pO", bufs=2, space="PSUM") as pO_pool,
            tc.tile_pool(name="pj", bufs=2, space="PSUM") as pj_pool,
            tc.tile_pool(name="att", bufs=5) as att_pool,
            tc.tile_pool(name="rec", bufs=4) as rec_pool,
            tc.tile_pool(name="osb", bufs=3) as osb_pool,
        ):
            # Dummy exp early: pulls the ~2.7us ACT table load off the
            # critical path (overlaps the initial DMA).
            warm = const_pool.tile([1, 8], f32, name="warm", tag="warm")
            nc.vector.memset(warm, 0.0)
            nc.scalar.activation(out=warm, in_=warm, func=EXP, scale=1.0)
            # ~3.5us of dummy matmuls while the input DMA runs: trips the HAM
            # activity window so the real stream starts at 2.4GHz, not 1.2
            wa = const_pool.tile([128, 512], bf16, name="wa", tag="wa")
            nc.vector.memset(wa, 0.0)
            for _ in range(9):
                pwarm = pj_pool.tile([128, 512], f32, name="pwarm", tag="pj")
                nc.tensor.matmul(pwarm, lhsT=wa[:, 0:128], rhs=wa, start=True, stop=True)

            # one DMA instruction per weight tensor: d-tile r lands at free-dim
            # group r of a single [128, KT*256] tile (issue cost on the sync
            # queue is ~0.6us per DMA instruction, so batching matters)
            wqa = const_pool.tile([128, KT * 256], bf16, name="wqa", tag="wqa")
            wka = const_pool.tile([128, KT * 256], bf16, name="wka", tag="wka")
            wva = const_pool.tile([128, KT * 256], bf16, name="wva", tag="wva")
            woa = const_pool.tile([128, NP * D], bf16, name="woa", tag="woa")
            nc.sync.dma_start(
                out=wva, in_=wv[:, :].rearrange("(r p) c -> p r c", r=KT)
            )
            xTall = big_pool.tile([128, KT * N], bf16, name="xTall", tag="xTall")
            xT = [xTall[:, N * r : N * (r + 1)] for r in range(KT)]
            xT4 = xTall.rearrange("p (r c) -> p r c", r=KT)
            # first column-slices of x^T land first so projections start early
            nc.sync.dma_start(
                out=xT4[:, :, 0:128],
                in_=xb[:, 0:128].rearrange("(r p) c -> p r c", r=KT),
            )
            nc.sync.dma_start(
                out=xT4[:, :, 128:512],
                in_=xb[:, 128:512].rearrange("(r p) c -> p r c", r=KT),
            )
            nc.sync.dma_start(
                out=xT4[:, :, 512:1024],
                in_=xb[:, 512:1024].rearrange("(r p) c -> p r c", r=KT),
            )
            nc.sync.dma_start(
                out=wqa, in_=wq[:, :].rearrange("(r p) c -> p r c", r=KT)
            )
            nc.sync.dma_start(
                out=wka, in_=wk[:, :].rearrange("(r p) c -> p r c", r=KT)
            )
            tri_sb = const_pool.tile([128, 128], bf16, name="tri_sb", tag="tri_sb")
            nc.sync.dma_start(out=tri_sb, in_=tri[:, :])
            nc.sync.dma_start(
                out=xT4[:, :, 1024:2048],
                in_=xb[:, 1024:2048].rearrange("(r p) c -> p r c", r=KT),
            )
            nc.sync.dma_start(
                out=woa, in_=wo[:, :].rearrange("(p q) c -> q p c", p=NP)
            )
            wq_sb = [wqa[:, 256 * r : 256 * (r + 1)] for r in range(KT)]
            wk_sb = [wka[:, 256 * r : 256 * (r + 1)] for r in range(KT)]
            wv_sb = [wva[:, 256 * r : 256 * (r + 1)] for r in range(KT)]
            wo_sb = [woa[:, D * p : D * (p + 1)] for p in range(NP)]

            qT, kT_ = [], []
            for p in range(NP):
                qT.append(big_pool.tile([128, N], bf16, name=f"qT{p}", tag=f"qT{p}"))
                kT_.append(big_pool.tile([128, N], bf16, name=f"kT{p}", tag=f"kT{p}"))
            v_all = big_pool.tile([128, NB * NP * VW], bf16, name="v_all", tag="v_all")
            # ones for the row-sum trick; data cols overwritten by vproj copies
            nc.vector.memset(v_all, 1.0)
            OT = []
            for p in range(NP):
                OT.append(big_pool.tile([128, N], bf16, name=f"OT{p}", tag=f"OT{p}"))

            va8 = v_all.rearrange("p (n g c) -> p n g c", n=NB, g=8)

            def vproj_stream(nbs):
                for nb in nbs:
                    pv = pj_pool.tile([128, HPC * DH], f32, name="pv", tag="pj")
                    for r in range(KT):
                        nc.tensor.matmul(
                            pv,
                            lhsT=xT[r][:, 128 * nb : 128 * (nb + 1)],
                            rhs=wv_sb[r],
                            start=(r == 0),
                            stop=(r == KT - 1),
                        )
                    # head h data -> 64-col group 2h+1 (odd groups; evens stay ones)
                    pv4 = pv.rearrange("p (h c) -> p h c", h=HPC)
                    nc.vector.tensor_copy(out=va8[:, nb, 1::2, :], in_=pv4)
                    yield

            def qkproj_stream(p, cs):
                for c in cs:
                    sl = slice(IC * c, IC * (c + 1))
                    for w_sb, dst in ((wq_sb, qT[p]), (wk_sb, kT_[p])):
                        pq = pj_pool.tile([128, IC], f32, name="pq", tag="pj")
                        for r in range(KT):
                            nc.tensor.matmul(
                                pq,
                                lhsT=w_sb[r][:, 128 * p : 128 * (p + 1)],
                                rhs=xT[r][:, sl],
                                start=(r == 0),
                                stop=(r == KT - 1),
                            )
                        nc.vector.tensor_copy(out=dst[:, sl], in_=pq)
                        yield

            def outproj_stream(nbs):
                for nb in nbs:
                    nsl = slice(128 * nb, 128 * (nb + 1))
                    for s in range(2):
                        po = pj_pool.tile([128, 512], f32, name="po", tag="pj")
                        for p in range(NP):
                            nc.tensor.matmul(
                                po,
                                lhsT=OT[p][:, nsl],
                                rhs=wo_sb[p][:, 512 * s : 512 * (s + 1)],
                                start=(p == 0),
                                stop=(p == NP - 1),
                            )
                        ob = osb_pool.tile([128, 512], bf16, name="ob", tag="osb")
                        # DVE only: a scalar.copy's sem-wait would head-of-line
                        # block the exp stream on the Scalar queue
                        nc.vector.tensor_copy(out=ob, in_=po)
                        nc.gpsimd.dma_start(out=outp[nsl, 512 * s : 512 * (s + 1)], in_=ob)
                        yield

            pend = []

            def drain(n):
                while len(pend) > n:
                    pend.pop(0)()

            def attention_stream():
                for cp in range(NCP):
                    for p in range(NP):
                        pO_A = pO_pool.tile([128, IC], f32, name=f"pOA{cp}{p}", tag="pO")
                        pO_B = pO_pool.tile([128, IC], f32, name=f"pOB{cp}{p}", tag="pO")
                        jmax = 4 * cp + 4
                        for jb in range(jmax):
                            o = max(0, 128 * jb - IC * cp)
                            jsl = slice(128 * jb, 128 * (jb + 1))
                            isl = slice(IC * cp + o, IC * (cp + 1))
                            pS = pS_pool.tile([128, 2 * IC], f32, name="pS", tag="pS")
                            pexp = att_pool.tile([128, 2 * IC], bf16, name="pexp", tag="pexp")
                            # S^T pair: K=64 each, concurrent via row groups
                            nc.tensor.matmul(
                                pS[:, o:IC],
                                lhsT=kT_[p][0:64, jsl],
                                rhs=qT[p][0:64, isl],
                                start=True,
                                stop=True,
                            )
                            nc.tensor.matmul(
                                pS[:, IC + o : 2 * IC],
                                lhsT=kT_[p][64:128, jsl],
                                rhs=qT[p][64:128, isl],
                                start=True,
                                stop=True,
                            )
                            # one exp for both heads: [128, 2, IC-o] strided AP
                            src = pS.rearrange("p (h w) -> p h w", h=2)[:, :, o:]
                            dst = pexp.rearrange("p (h w) -> p h w", h=2)[:, :, o:]
                            nc.scalar.activation(out=dst, in_=src, func=EXP, scale=SCALE)
                            if 128 * jb >= IC * cp:  # diagonal block: 0/1 mask
                                for half in range(2):
                                    hb = IC * half
                                    nc.vector.tensor_mul(
                                        pexp[:, hb + o : hb + o + 128],
                                        pexp[:, hb + o : hb + o + 128],
                                        tri_sb,
                                    )

                            def av_unit(p=p, jb=jb, o=o, jmax=jmax, pO_A=pO_A, pO_B=pO_B, pexp=pexp):
                                vo = 2 * VW * jb + VW * p
                                nc.tensor.matmul(
                                    pO_A[:, o:IC],
                                    lhsT=v_all[:, vo : vo + 128],
                                    rhs=pexp[:, o:IC],
                                    start=(jb == 0),
                                    stop=(jb == jmax - 1),
                                    skip_group_check=True,
                                )
                                nc.tensor.matmul(
                                    pO_B[:, o:IC],
                                    lhsT=v_all[:, vo + 128 : vo + 256],
                                    rhs=pexp[:, IC + o : 2 * IC],
                                    start=(jb == 0),
                                    stop=(jb == jmax - 1),
                                    skip_group_check=True,
                                )

                            pend.append(av_unit)
                            drain(DELAY)
                            yield

                        # normalize straight from PSUM; OT written in bf16
                        csl = slice(IC * cp, IC * (cp + 1))
                        rec_A = rec_pool.tile([64, IC], f32, name="recA", tag="rec")
                        rec_B = rec_pool.tile([64, IC], f32, name="recB", tag="rec")

                        def recip_a(pO_A=pO_A, rec_A=rec_A):
                            nc.vector.reciprocal_approx_fast(out=rec_A, in_=pO_A[0:64, :])

                        def mul_a(pO_A=pO_A, rec_A=rec_A, p=p, csl=csl):
                            nc.vector.tensor_mul(OT[p][0:64, csl], pO_A[64:128, :], rec_A)

                        def recip_b(pO_B=pO_B, rec_B=rec_B):
                            nc.vector.reciprocal_approx_fast(out=rec_B, in_=pO_B[0:64, :])

                        def mul_b(pO_B=pO_B, rec_B=rec_B, p=p, csl=csl):
                            nc.vector.tensor_mul(OT[p][64:128, csl], pO_B[64:128, :], rec_B)

                        pend.append(recip_a)
                        pend.append(mul_a)
                        pend.append(recip_b)
                        pend.append(mul_b)

            # ---- prologue: everything attention (cp0, pair 0) needs ----
            for _ in vproj_stream(range(0, 4)):
                pass
            for _ in qkproj_stream(0, [0]):
                pass

            # ---- gated fillers pulled between attention steps ----
            fillers = [
                (0, qkproj_stream(1, [0])),
                (0, qkproj_stream(0, [1])),
                (0, qkproj_stream(1, [1])),
                (0, vproj_stream(range(4, 8))),
                (8, qkproj_stream(0, [2])),
                (8, qkproj_stream(1, [2])),
                (8, vproj_stream(range(8, 12))),
                (12, outproj_stream(range(0, 4))),
                (24, qkproj_stream(0, [3])),
                (24, qkproj_stream(1, [3])),
                (24, vproj_stream(range(12, 16))),
                (48, outproj_stream(range(4, 8))),
                (64, outproj_stream(range(8, 12))),
            ]

            def pull_filler(step):
                for i, (gate, gen) in enumerate(fillers):
                    if step < gate:
                        continue
                    if next(gen, "end") == "end":
                        fillers.pop(i)
                        continue
                    return True
                return False

            att = attention_stream()
            for step, _ in enumerate(att):
                pull_filler(step)
                if step < 2 or step >= 48:
                    pull_filler(step)
            drain(0)
            for _, gen in fillers:
                for _ in gen:
                    pass
            # pair 1's cp3 normalize is only issued by drain(0) above, so the
            # last output-projection blocks must stay in the epilogue. Stage
            # them in one wide SBUF tile and ship with a single DMA (per-DMA
            # issue costs ~0.6us on the queue, the dominant tail cost).
            obig = big_pool.tile([128, 4 * D], bf16, name="obig", tag="obig")
            for i, nb in enumerate(range(12, 16)):
                nsl = slice(128 * nb, 128 * (nb + 1))
                for s in range(2):
                    po = pj_pool.tile([128, 512], f32, name="po", tag="pj")
                    for p in range(NP):
                        nc.tensor.matmul(
                            po,
                            lhsT=OT[p][:, nsl],
                            rhs=wo_sb[p][:, 512 * s : 512 * (s + 1)],
                            start=(p == 0),
                            stop=(p == NP - 1),
                        )
                    dst = obig[:, D * i + 512 * s : D * i + 512 * (s + 1)]
                    if s == 0:
                        nc.vector.tensor_copy(out=dst, in_=po)
                    else:
                        nc.scalar.copy(out=dst, in_=po)
                nc.sync.dma_start(
                    out=outp[128 * nb : 128 * (nb + 1), :],
                    in_=obig[:, D * i : D * (i + 1)],
                )

    nc.compile()
    return nc


def kernel(x, mask, Wq, Wkv, Wout, b_out):
    global _last_results
    from concourse.bass_utils import run_bass_kernel_spmd

    bf = ml_dtypes.bfloat16
    x = np.asarray(x, dtype=np.float32)
    Wq = np.asarray(Wq, dtype=np.float32)
    Wkv = np.asarray(Wkv, dtype=np.float32)
    Wout = np.asarray(Wout, dtype=np.float32)
    b_out = np.asarray(b_out, dtype=np.float32)

    if "nc" not in _cached:
        _cached["nc"] = _build_program()
    nc = _cached["nc"]

    jj, ii = np.mgrid[0:128, 0:128]
    # pexp[j, o+c] is masked (multiplied by 0) where j > c
    tri = (jj <= ii).astype(np.float32).astype(bf)

    xTs = [np.ascontiguousarray(x[b].T).astype(bf) for b in range(B)]

    in_maps = []
    for c in range(NCORES):
        b = c // 4
        h0 = HPC * (c % 4)
        in_maps.append(
            {
                "xb": xTs[b],
                "wq": np.ascontiguousarray(Wq[:, DH * h0 : DH * (h0 + HPC)]).astype(bf),
                "wk": np.ascontiguousarray(Wkv[:, DH * h0 : DH * (h0 + HPC)]).astype(bf),
                "wv": np.ascontiguousarray(Wkv[:, D + DH * h0 : D + DH * (h0 + HPC)]).astype(bf),
                "wo": np.ascontiguousarray(Wout[DH * h0 : DH * (h0 + HPC), :]).astype(bf),
                "tri": tri,
            }
        )

    res = run_bass_kernel_spmd(
        nc,
        in_maps,
        core_ids=list(range(NCORES)),
        trace=bool(int(os.environ.get("KERNEL_TRACE", "0"))),
    )
    _last_results = res
    parts = [r["outp"] for r in res.results]
    out = np.empty((B, N, D), dtype=np.float32)
    for b in range(B):
        acc = parts[4 * b].astype(np.float32).copy()
        for c in range(4 * b + 1, 4 * b + 4):
            acc += parts[c]
        out[b] = acc + b_out[None, :]
    return out
